# revision 33
# baseline (speedup 1.0000x reference)
"""Causal self-attention Trainium2 kernel (B=2, T=2048, D=1024, H=16).

Sharding: 8 cores = 2 batch groups x 4 head groups; each core computes
batch b = c//4, heads 4*(c%4)..4*(c%4)+3 (256 QKV dims), and a partial
output projection y_cT = W_o[:, slice] @ attnout (contribution summed on
host across the 4 cores of each batch group).

All on-device compute in fp16 operands with fp32 PSUM accumulation.
Everything is kept "transposed" ([dim, seq]) so no on-device transposes
are needed:
  QT/KT = W @ xT               [256, 2048]
  V     = x @ WvT              [2048, 256]   (seq on partitions)
  ST[k,q] = sum_d K[k,d]Q[q,d] (k on partitions, q streaming)
  causal mask applied additively on PSUM (Pool engine) before exp
  P = exp(ST/8)  (ACT engine)
  avT[d,q] = sum_k [V|1][k,d] P[k,q]  -> row of ones gives softmax denom
  attnout[d,q] = avT * (1/denom); denom reciprocal broadcast across
  partitions via Pool partition_broadcast
  yT[e,q] = WoT.T @ attnout  (partial over this core's 256 dims)

Schedule: DMAs are split/prioritized so the first projection starts ~1us
in; V projections and the previous block's output projection are deferred
into later (ACT-bound) attention blocks as PE filler work.
"""

import numpy as np

import concourse.bass as bass
import concourse.mybir as mybir
from concourse.tile import TileContext
from concourse.vector_clock import ScopedClock
from concourse.bass_utils import run_bass_kernel_spmd

B, T, D = 2, 2048, 1024
H, DK = 16, 64
NCORES = 8
HPC = 4            # heads per core
QB = 512           # q block size
NQB = T // QB      # 4
NKC = T // 128     # 16 k-chunks
F16 = mybir.dt.float16
F32 = mybir.dt.float32
EXPF = mybir.ActivationFunctionType.Exp
MASK_NEG = -30000.0


class TC(TileContext):
    """This container's walrus only accepts one sync-wait per TPB_CTRL
    instruction; split the tile tail-drain waits into one nop each."""

    def _drain_and_barrier(self, tick_clock, wait_clock):
        carrier = self.nc.sync.nop(nofuse=True)
        wait_clock.add_sem_waits(
            carrier.ins, ScopedClock({None: tick_clock.global_clock})
        )
        si = carrier.ins.sync_info
        if si is not None and len(si.on_wait) > 1:
            waits = list(si.on_wait)
            carrier.ins.sync_info = mybir.SyncInfo(
                on_wait=[waits[0]], on_update=list(si.on_update)
            )
            for w in waits[1:]:
                nop = self.nc.sync.nop(nofuse=True)
                nop.ins.sync_info = mybir.SyncInfo(on_wait=[w], on_update=[])
        self.nc.sync.drain()
        self.nc.all_engine_barrier()
        assert self.sems is not None
        popped = self.nc._tile_sem_poison_stack.pop()
        assert popped is self._sem_poison
        self.nc.clear_and_free_semaphores(list(self.sems.allocated().values()))
        self.nc.all_engine_barrier()


def split_multi_waits(nc):
    """This walrus build accepts only one sync-wait per instruction; hoist
    extra waits onto single-wait NoOps inserted just before the instruction
    on the same engine."""
    for fn in nc.m.functions:
        for bb in fn.blocks:
            out = []
            for ins in bb.instructions:
                si = getattr(ins, "sync_info", None)
                is_isa = "ISA" in type(ins).__name__ or "PartitionBroadcast" in type(ins).__name__
                keep = 0 if is_isa else 1
                if si is not None and len(si.on_wait) > keep:
                    waits = list(si.on_wait)
                    keep_waits = waits[len(waits) - keep :] if keep else []
                    for i, w in enumerate(waits[: len(waits) - keep]):
                        out.append(
                            mybir.InstNoOp(
                                name=f"{ins.name}_w{i}",
                                engine=ins.engine,
                                sync_info=mybir.SyncInfo(on_wait=[w], on_update=[]),
                                bass_nofuse=True,
                            )
                        )
                    ins.sync_info = mybir.SyncInfo(
                        on_wait=keep_waits, on_update=list(si.on_update)
                    )
                out.append(ins)
            bb.instructions = out


def build_nc():
    nc = bass.Bass("TRN2", target_bir_lowering=False, debug=False)
    xT = nc.dram_tensor("xT", [D, T], F16, kind="ExternalInput")
    wqT = nc.dram_tensor("wqT", [128, 2048], F16, kind="ExternalInput")
    wkT = nc.dram_tensor("wkT", [128, 2048], F16, kind="ExternalInput")
    wvT = nc.dram_tensor("wvT", [128, 2048], F16, kind="ExternalInput")
    woT = nc.dram_tensor("woT", [128, 2048], F16, kind="ExternalInput")
    tmask = nc.dram_tensor("tmask", [128, 128], F16, kind="ExternalInput")
    sel65 = nc.dram_tensor("sel65", [65, 128], F16, kind="ExternalInput")
    yT = nc.dram_tensor("yT", [D, T], F16, kind="ExternalOutput")

    with TC(nc) as tc:
        with (
            tc.tile_pool(name="const", bufs=1) as cpool,
            tc.tile_pool(name="work", bufs=2) as wpool,
            tc.tile_pool(name="psA", bufs=2, space="PSUM") as psA,
            tc.tile_pool(name="psS", bufs=2, space="PSUM") as psS,
            tc.tile_pool(name="psV", bufs=1, space="PSUM") as psV,
        ):
            # ---- tiles ----
            xt = [cpool.tile([128, T], F16, tag=f"xt{kc}", name=f"xt{kc}") for kc in range(8)]
            wq = cpool.tile([128, 2048], F16, tag="wq", name="wq")
            wk = cpool.tile([128, 2048], F16, tag="wk", name="wk")
            wv = cpool.tile([128, 2048], F16, tag="wv", name="wv")
            wo = cpool.tile([128, 2048], F16, tag="wo", name="wo")
            maskt = cpool.tile([128, 128], F16, tag="mask")
            selt = cpool.tile([65, 128], F16, tag="sel")

            # warm the ACT exp table while DMAs run
            warm = cpool.tile([1, 8], F32, tag="warm")
            nc.vector.memset(warm[:], 0.0)
            nc.scalar.activation(warm[:], warm[:], EXPF, scale=1.0)

            # ---- input DMAs split across the two HWDGE queues (SP and
            # ACT): urgent first-projection inputs on SP, the rest on ACT ----
            def xf(kc):
                nc.sync.dma_start(
                    xt[kc][:, 0:QB], xT[128 * kc : 128 * (kc + 1), 0:QB]
                )

            nc.sync.dma_start(wq[:, 0:1024], wqT[:, 0:1024])
            xf(0)
            xf(1)
            nc.sync.dma_start(wq[:, 1024:2048], wqT[:, 1024:2048])
            xf(2)
            nc.sync.dma_start(wk[:], wkT[:, :])
            for kc in range(3, 8):
                xf(kc)
            nc.sync.dma_start(maskt[:], tmask[:, :])
            nc.sync.dma_start(selt[:], sel65[:, :])
            nc.sync.dma_start(wv[:], wvT[:, :])
            for kc in range(8):
                nc.sync.dma_start(
                    xt[kc][:, QB:T], xT[128 * kc : 128 * (kc + 1), QB:T]
                )
            nc.sync.dma_start(wo[:], woT[:, :])

            qt = [cpool.tile([128, T], F16, tag=f"qt{p}", name=f"qt{p}") for p in range(2)]
            kt = [cpool.tile([128, T], F16, tag=f"kt{p}", name=f"kt{p}") for p in range(2)]
            ao = [cpool.tile([128, T], F16, tag=f"ao{p}", name=f"ao{p}") for p in range(2)]
            vp = [
                [cpool.tile([128, 193], F16, tag=f"vp{tt}_{p}", name=f"vp{tt}_{p}") for p in range(2)]
                for tt in range(NKC)
            ]

            # ---- Q, K projections: out[p][:, jq] = W.T @ xT ----
            def qk_proj(p, wt, out_t, jq):
                ps = psA.tile([128, QB], F32, tag="psA", name=f"psqk{p}{jq}")
                for kc in range(8):
                    nc.tensor.matmul(
                        ps[:],
                        wt[:, 256 * kc + 128 * p : 256 * kc + 128 * (p + 1)],
                        xt[kc][:, QB * jq : QB * (jq + 1)],
                        start=(kc == 0),
                        stop=(kc == 7),
                    )
                nc.scalar.copy(out_t[p][:, QB * jq : QB * (jq + 1)], ps[:])

            def v_proj(tt):
                ps = psA.tile([128, QB], F32, tag="psA", name=f"psv{tt}")
                for kc in range(8):
                    nc.tensor.matmul(
                        ps[:, 0:256],
                        xt[kc][:, 128 * tt : 128 * (tt + 1)],
                        wv[:, 256 * kc : 256 * (kc + 1)],
                        start=(kc == 0),
                        stop=(kc == 7),
                    )
                for p in range(2):
                    v = vp[tt][p]
                    nc.gpsimd.memset(v[:, 64:66], 1.0)
                    nc.gpsimd.memset(v[:, 66:129], 0.0)
                    nc.vector.tensor_copy(v[:, 0:64], ps[:, 128 * p : 128 * p + 64])
                    nc.vector.tensor_copy(v[:, 129:193], ps[:, 128 * p + 64 : 128 * p + 128])

            # ---- output projection for one (q block, dim chunk) ----
            def out_proj_unit(j, et, tail=False):
                ps = psA.tile([128, QB], F32, tag="psA", name=f"pso{j}{et}")
                for p in range(2):
                    nc.tensor.matmul(
                        ps[:],
                        wo[:, 1024 * p + 128 * et : 1024 * p + 128 * (et + 1)],
                        ao[p][:, QB * j : QB * (j + 1)],
                        start=(p == 0),
                        stop=(p == 1),
                    )
                ysb = wpool.tile([128, QB], F16, tag="ysb", bufs=3, name="ysb")
                nc.vector.tensor_copy(ysb[:], ps[:])
                ring = nc.scalar if (tail and et % 2) else nc.sync
                ring.dma_start(
                    yT[128 * et : 128 * (et + 1), QB * j : QB * (j + 1)], ysb[:]
                )

            # reciprocal staging tiles: rows 1..63 must never hold NaN (the
            # sel matmul multiplies them by zero); zero them once up front
            rcbs = [
                cpool.tile([65, QB], F16, tag=f"rcb{i}", name=f"rcb{i}")
                for i in range(2)
            ]
            for t in rcbs:
                nc.gpsimd.memset(t[:], 0.0)
            rci = [0]

            # ---- attention over one q block; fillers are (weight, closure)
            # PE work consumed pro-rata across chunk steps; pre_units are
            # placed before the scores of a specific (p, kc) chunk ----
            def attention(j, fillers, pre_units):
                quota = fillers
                total_w = sum(w for w, _ in quota) or 1
                done_w = 0
                fi = 0
                nch = 4 * j + 4
                steps = 2 * nch
                step = 0

                def scores(p, kc):
                    off = max(0, 128 * (kc - 4 * j))
                    ps = psS.tile([128, 1024], F32, tag="psS", name="psS")
                    for h in range(2):
                        nc.tensor.matmul(
                            ps[:, 512 * h + off : 512 * (h + 1)],
                            kt[p][64 * h : 64 * (h + 1), 128 * kc : 128 * (kc + 1)],
                            qt[p][64 * h : 64 * (h + 1), QB * j + off : QB * (j + 1)],
                            start=True,
                            stop=True,
                        )
                    return ps, off

                for p in range(2):
                    P = wpool.tile([128, 1024 * NKC], F16, tag="P", bufs=2, name="P")
                    av0 = psV.tile([65, QB], F32, tag="av0", name="av0")
                    av1 = psV.tile([128, QB], F32, tag="av1", name="av1")
                    for u in pre_units.pop((p, 0), ()):
                        u()
                    ps, off = scores(p, 0)
                    rcb = rcbs[rci[0]]
                    rci[0] ^= 1
                    bcs = wpool.tile([128, QB], F32, tag="bcs", bufs=2, name="bcs")

                    def norm_slice(s):
                        # denominators for q columns [128s, 128s+128) are
                        # final once diagonal chunk 4j+s has accumulated
                        sl = slice(128 * s, 128 * (s + 1))
                        with nc.allow_low_precision("softmax denom recip fp16"):
                            nc.vector.reciprocal(rcb[64:65, sl], av0[64:65, sl])
                            nc.vector.reciprocal(rcb[0:1, sl], av1[0:1, sl])
                        bct = psA.tile([128, 128], F32, tag="psA", name=f"bc{j}{s}")
                        nc.tensor.matmul(
                            bct[:], selt[:], rcb[:, sl], start=True, stop=True
                        )
                        nc.vector.tensor_copy(bcs[:, sl], bct[:])
                        gsl = slice(QB * j + 128 * s, QB * j + 128 * (s + 1))
                        nc.vector.tensor_mul(ao[p][0:64, gsl], av0[0:64, sl], bcs[0:64, sl])
                        nc.vector.tensor_mul(
                            ao[p][64:128, gsl], av1[64:128, sl], bcs[64:128, sl]
                        )

                    for kc in range(nch):
                        nc.scalar.activation(
                            P[:, 1024 * kc + off : 1024 * (kc + 1)],
                            ps[:, off:1024],
                            EXPF,
                            scale=0.125,
                        )
                        if kc >= 4 * j:  # diagonal band: mask 128x128 blocks
                            for h in range(2):
                                sl = slice(
                                    1024 * kc + 512 * h + off,
                                    1024 * kc + 512 * h + off + 128,
                                )
                                nc.gpsimd.tensor_mul(P[:, sl], P[:, sl], maskt[:])
                        # next chunk's scores keep PE busy while ACT runs exp
                        if kc + 1 < nch:
                            for u in pre_units.pop((p, kc + 1), ()):
                                u()
                            ps_n, off_n = scores(p, kc + 1)
                        step += 1
                        while fi < len(quota) and done_w * steps < total_w * step:
                            done_w += quota[fi][0]
                            quota[fi][1]()
                            fi += 1
                        nc.tensor.matmul(
                            av0[:, off:QB],
                            vp[kc][p][:, 0:65],
                            P[:, 1024 * kc + off : 1024 * kc + 512],
                            start=(kc == 0),
                            stop=(kc == nch - 1),
                        )
                        nc.tensor.matmul(
                            av1[:, off:QB],
                            vp[kc][p][:, 65:193],
                            P[:, 1024 * kc + 512 + off : 1024 * (kc + 1)],
                            start=(kc == 0),
                            stop=(kc == nch - 1),
                        )
                        if kc > 4 * j:  # one chunk of slack for the recip
                            norm_slice(kc - 4 * j - 1)
                        if kc + 1 < nch:
                            ps, off = ps_n, off_n
                    norm_slice(3)
                for w, u in fillers[len(quota) :]:
                    u()

            # ---- main schedule: pair-0 projections first so attention(0)
            # can start after two projection calls ----
            for pp, wt, ot in ((0, wq, qt), (0, wk, kt), (1, wq, qt), (1, wk, kt)):
                qk_proj(pp, wt, ot, 0)
            for j in range(NQB):
                fillers = []
                if j + 1 < NQB:
                    jq = j + 1
                    for pp, wt, ot in ((0, wq, qt), (0, wk, kt), (1, wq, qt), (1, wk, kt)):
                        fillers.append(
                            (4096, lambda pp=pp, wt=wt, ot=ot, jq=jq: qk_proj(pp, wt, ot, jq))
                        )
                if j >= 1:
                    for et in range(8):
                        fillers.append((1024, lambda j=j, et=et: out_proj_unit(j - 1, et)))
                pre_units = {}
                for tt in range(4 * j, 4 * j + 4):
                    pre_units.setdefault((0, max(0, tt - 1)), []).append(
                        lambda tt=tt: v_proj(tt)
                    )
                attention(j, fillers, pre_units)
            for et in range(8):
                out_proj_unit(NQB - 1, et, tail=True)
    split_multi_waits(nc)
    return nc


_NC = None


def _get_nc():
    global _NC
    if _NC is None:
        _NC = build_nc()
    return _NC


def kernel(x, W_q, W_k, W_v, W_o):
    x = np.asarray(x, dtype=np.float32)
    W_q = np.asarray(W_q, dtype=np.float32)
    W_k = np.asarray(W_k, dtype=np.float32)
    W_v = np.asarray(W_v, dtype=np.float32)
    W_o = np.asarray(W_o, dtype=np.float32)

    def pack8(a):  # [1024, 256] -> [128, 8*256], chunk kc at cols 256*kc
        return np.ascontiguousarray(
            a.reshape(8, 128, 256).transpose(1, 0, 2).reshape(128, 2048)
        ).astype(np.float16)

    def pack2(a):  # [256, 1024] -> [128, 2*1024], chunk p at cols 1024*p
        return np.ascontiguousarray(
            a.reshape(2, 128, 1024).transpose(1, 0, 2).reshape(128, 2048)
        ).astype(np.float16)

    tmask = np.triu(np.ones((128, 128), dtype=np.float16))
    sel65 = np.zeros((65, 128), dtype=np.float16)
    sel65[64, 0:64] = 1.0
    sel65[0, 64:128] = 1.0
    xTb = [np.ascontiguousarray(x[b].T).astype(np.float16) for b in range(B)]
    in_maps = []
    for c in range(NCORES):
        b, g = c // 4, c % 4
        hs = 256 * g
        in_maps.append(
            {
                "xT": xTb[b],
                "wqT": pack8(W_q[hs : hs + 256, :].T),
                "wkT": pack8(W_k[hs : hs + 256, :].T),
                "wvT": pack8(W_v[hs : hs + 256, :].T),
                "woT": pack2(W_o[:, hs : hs + 256].T),
                "tmask": tmask,
                "sel65": sel65,
            }
        )
    res = run_bass_kernel_spmd(_get_nc(), in_maps, core_ids=list(range(NCORES)))
    out = np.empty((B, T, D), dtype=np.float32)
    for b in range(B):
        acc = res.results[4 * b]["yT"].astype(np.float32)
        for g in range(1, 4):
            acc = acc + res.results[4 * b + g]["yT"]
        out[b] = acc.T
    return out


# revision 49
# speedup vs baseline: 1.0192x; 1.0192x over previous
"""Causal self-attention Trainium2 kernel (B=2, T=2048, D=1024, H=16).

Sharding: 8 cores = 2 batch groups x 4 head groups; each core computes
batch b = c//4, heads 4*(c%4)..4*(c%4)+3 (256 QKV dims), and a partial
output projection y_cT = W_o[:, slice] @ attnout (contribution summed on
host across the 4 cores of each batch group).

All on-device compute in fp16 operands with fp32 PSUM accumulation.
Everything is kept "transposed" ([dim, seq]) so no on-device transposes
are needed:
  QT/KT = W @ xT               [256, 2048]
  V     = x @ WvT              [2048, 256]   (seq on partitions)
  ST[k,q] = sum_d K[k,d]Q[q,d] (k on partitions, q streaming)
  causal mask applied additively on PSUM (Pool engine) before exp
  P = exp(ST/8)  (ACT engine)
  avT[d,q] = sum_k [V|1][k,d] P[k,q]  -> row of ones gives softmax denom
  attnout[d,q] = avT * (1/denom); denom reciprocal broadcast across
  partitions via Pool partition_broadcast
  yT[e,q] = WoT.T @ attnout  (partial over this core's 256 dims)

Schedule: DMAs are split/prioritized so the first projection starts ~1us
in; V projections and the previous block's output projection are deferred
into later (ACT-bound) attention blocks as PE filler work.
"""

import numpy as np

import concourse.bass as bass
import concourse.mybir as mybir
from concourse.tile import TileContext
from concourse.vector_clock import ScopedClock
from concourse.bass_utils import run_bass_kernel_spmd

B, T, D = 2, 2048, 1024
H, DK = 16, 64
NCORES = 8
HPC = 4            # heads per core
QB = 512           # q block size
NQB = T // QB      # 4
NKC = T // 128     # 16 k-chunks
F16 = mybir.dt.float16
F32 = mybir.dt.float32
EXPF = mybir.ActivationFunctionType.Exp
MASK_NEG = -30000.0


class TC(TileContext):
    """This container's walrus only accepts one sync-wait per TPB_CTRL
    instruction; split the tile tail-drain waits into one nop each."""

    def _drain_and_barrier(self, tick_clock, wait_clock):
        carrier = self.nc.sync.nop(nofuse=True)
        wait_clock.add_sem_waits(
            carrier.ins, ScopedClock({None: tick_clock.global_clock})
        )
        si = carrier.ins.sync_info
        if si is not None and len(si.on_wait) > 1:
            waits = list(si.on_wait)
            carrier.ins.sync_info = mybir.SyncInfo(
                on_wait=[waits[0]], on_update=list(si.on_update)
            )
            for w in waits[1:]:
                nop = self.nc.sync.nop(nofuse=True)
                nop.ins.sync_info = mybir.SyncInfo(on_wait=[w], on_update=[])
        self.nc.sync.drain()
        self.nc.all_engine_barrier()
        assert self.sems is not None
        popped = self.nc._tile_sem_poison_stack.pop()
        assert popped is self._sem_poison
        self.nc.clear_and_free_semaphores(list(self.sems.allocated().values()))
        self.nc.all_engine_barrier()


def split_multi_waits(nc):
    """This walrus build accepts only one sync-wait per instruction; hoist
    extra waits onto single-wait NoOps inserted just before the instruction
    on the same engine."""
    for fn in nc.m.functions:
        for bb in fn.blocks:
            out = []
            for ins in bb.instructions:
                si = getattr(ins, "sync_info", None)
                is_isa = "ISA" in type(ins).__name__ or "PartitionBroadcast" in type(ins).__name__
                keep = 0 if is_isa else 1
                if si is not None and len(si.on_wait) > keep:
                    waits = list(si.on_wait)
                    keep_waits = waits[len(waits) - keep :] if keep else []
                    for i, w in enumerate(waits[: len(waits) - keep]):
                        out.append(
                            mybir.InstNoOp(
                                name=f"{ins.name}_w{i}",
                                engine=ins.engine,
                                sync_info=mybir.SyncInfo(on_wait=[w], on_update=[]),
                                bass_nofuse=True,
                            )
                        )
                    ins.sync_info = mybir.SyncInfo(
                        on_wait=keep_waits, on_update=list(si.on_update)
                    )
                out.append(ins)
            bb.instructions = out


def build_nc():
    nc = bass.Bass("TRN2", target_bir_lowering=False, debug=False)
    xT = nc.dram_tensor("xT", [D, T], F16, kind="ExternalInput")
    wqT = nc.dram_tensor("wqT", [128, 2048], F16, kind="ExternalInput")
    wkT = nc.dram_tensor("wkT", [128, 2048], F16, kind="ExternalInput")
    wvT = nc.dram_tensor("wvT", [128, 2048], F16, kind="ExternalInput")
    woT = nc.dram_tensor("woT", [128, 2048], F16, kind="ExternalInput")
    tmask = nc.dram_tensor("tmask", [128, 128], F16, kind="ExternalInput")
    sel65 = nc.dram_tensor("sel65", [65, 128], F16, kind="ExternalInput")
    yT = nc.dram_tensor("yT", [D, T], F16, kind="ExternalOutput")

    with TC(nc) as tc:
        with (
            tc.tile_pool(name="const", bufs=1) as cpool,
            tc.tile_pool(name="work", bufs=2) as wpool,
            tc.tile_pool(name="psA", bufs=2, space="PSUM") as psA,
            tc.tile_pool(name="psS", bufs=2, space="PSUM") as psS,
            tc.tile_pool(name="psV", bufs=1, space="PSUM") as psV,
        ):
            # ---- tiles ----
            xt = [cpool.tile([128, T], F16, tag=f"xt{kc}", name=f"xt{kc}") for kc in range(8)]
            wq = cpool.tile([128, 2048], F16, tag="wq", name="wq")
            wk = cpool.tile([128, 2048], F16, tag="wk", name="wk")
            wv = cpool.tile([128, 2048], F16, tag="wv", name="wv")
            wo = cpool.tile([128, 2048], F16, tag="wo", name="wo")
            maskt = cpool.tile([128, 128], F16, tag="mask")
            selt = cpool.tile([65, 128], F16, tag="sel")

            # warm the ACT exp table while DMAs run
            warm = cpool.tile([1, 8], F32, tag="warm")
            nc.vector.memset(warm[:], 0.0)
            nc.scalar.activation(warm[:], warm[:], EXPF, scale=1.0)

            # ---- input DMAs split across the two HWDGE queues (SP and
            # ACT): urgent first-projection inputs on SP, the rest on ACT ----
            def xf(kc):
                nc.sync.dma_start(
                    xt[kc][:, 0:QB], xT[128 * kc : 128 * (kc + 1), 0:QB]
                )

            nc.sync.dma_start(wq[:, 0:1024], wqT[:, 0:1024])
            xf(0)
            xf(1)
            nc.sync.dma_start(wq[:, 1024:2048], wqT[:, 1024:2048])
            xf(2)
            nc.sync.dma_start(wk[:], wkT[:, :])
            for kc in range(3, 8):
                xf(kc)
            nc.sync.dma_start(maskt[:], tmask[:, :])
            nc.sync.dma_start(selt[:], sel65[:, :])
            nc.sync.dma_start(wv[:], wvT[:, :])
            for kc in range(8):
                nc.sync.dma_start(
                    xt[kc][:, QB:T], xT[128 * kc : 128 * (kc + 1), QB:T]
                )
            nc.sync.dma_start(wo[:], woT[:, :])

            qt = [cpool.tile([128, T], F16, tag=f"qt{p}", name=f"qt{p}") for p in range(2)]
            kt = [cpool.tile([128, T], F16, tag=f"kt{p}", name=f"kt{p}") for p in range(2)]
            ao = [cpool.tile([128, T], F16, tag=f"ao{p}", name=f"ao{p}") for p in range(2)]
            vp = [
                [cpool.tile([128, 193], F16, tag=f"vp{tt}_{p}", name=f"vp{tt}_{p}") for p in range(2)]
                for tt in range(NKC)
            ]

            # ---- Q, K projections: out[p][:, jq] = W.T @ xT ----
            def qk_proj(p, wt, out_t, jq):
                ps = psA.tile([128, QB], F32, tag="psA", name=f"psqk{p}{jq}")
                for kc in range(8):
                    nc.tensor.matmul(
                        ps[:],
                        wt[:, 256 * kc + 128 * p : 256 * kc + 128 * (p + 1)],
                        xt[kc][:, QB * jq : QB * (jq + 1)],
                        start=(kc == 0),
                        stop=(kc == 7),
                    )
                nc.scalar.copy(out_t[p][:, QB * jq : QB * (jq + 1)], ps[:])

            def v_proj(tt):
                ps = psA.tile([128, QB], F32, tag="psA", name=f"psv{tt}")
                for kc in range(8):
                    nc.tensor.matmul(
                        ps[:, 0:256],
                        xt[kc][:, 128 * tt : 128 * (tt + 1)],
                        wv[:, 256 * kc : 256 * (kc + 1)],
                        start=(kc == 0),
                        stop=(kc == 7),
                    )
                for p in range(2):
                    v = vp[tt][p]
                    nc.gpsimd.memset(v[:, 64:66], 1.0)
                    nc.gpsimd.memset(v[:, 66:129], 0.0)
                    nc.vector.tensor_copy(v[:, 0:64], ps[:, 128 * p : 128 * p + 64])
                    nc.vector.tensor_copy(v[:, 129:193], ps[:, 128 * p + 64 : 128 * p + 128])

            # ---- output projection for one (q block, dim chunk) ----
            def out_proj_unit(j, et, tail=False):
                ps = psA.tile([128, QB], F32, tag="psA", name=f"pso{j}{et}")
                for p in range(2):
                    nc.tensor.matmul(
                        ps[:],
                        wo[:, 1024 * p + 128 * et : 1024 * p + 128 * (et + 1)],
                        ao[p][:, QB * j : QB * (j + 1)],
                        start=(p == 0),
                        stop=(p == 1),
                    )
                ysb = wpool.tile([128, QB], F16, tag="ysb", bufs=8, name="ysb")
                if tail and et % 2:
                    nc.scalar.copy(ysb[:], ps[:])
                else:
                    nc.vector.tensor_copy(ysb[:], ps[:])
                ring = nc.scalar if (tail and et % 2) else nc.sync
                ring.dma_start(
                    yT[128 * et : 128 * (et + 1), QB * j : QB * (j + 1)], ysb[:]
                )

            # reciprocal staging tiles: rows 1..63 must never hold NaN (the
            # sel matmul multiplies them by zero); zero them once up front
            rcbs = [
                cpool.tile([65, QB], F16, tag=f"rcb{i}", name=f"rcb{i}")
                for i in range(2)
            ]
            for t in rcbs:
                nc.gpsimd.memset(t[:], 0.0)
            rci = [0]

            # ---- attention over one q block; fillers are (weight, closure)
            # PE work consumed pro-rata across chunk steps; pre_units are
            # placed before the scores of a specific (p, kc) chunk ----
            def attention(j, fillers, pre_units):
                total_w = sum(w for w, _ in fillers) or 1
                done_w = 0
                fi = 0
                nch = 4 * j + 4
                steps = 2 * nch
                step = 0

                def scores(p, kc):
                    off = max(0, 128 * (kc - 4 * j))
                    ps = psS.tile([128, 1024], F32, tag="psS", name="psS")
                    for h in range(2):
                        nc.tensor.matmul(
                            ps[:, 512 * h + off : 512 * (h + 1)],
                            kt[p][64 * h : 64 * (h + 1), 128 * kc : 128 * (kc + 1)],
                            qt[p][64 * h : 64 * (h + 1), QB * j + off : QB * (j + 1)],
                            start=True,
                            stop=True,
                        )
                    return ps, off

                prefetched = [None]
                for p in range(2):
                    P = wpool.tile([128, 1024 * NKC], F16, tag="P", bufs=2, name="P")
                    av0 = psV.tile([65, QB], F32, tag="av0", name="av0")
                    av1 = psV.tile([128, QB], F32, tag="av1", name="av1")
                    if prefetched[0] is not None:
                        ps, off = prefetched[0]
                        prefetched[0] = None
                    else:
                        for u in pre_units.pop((p, 0), ()):
                            u()
                        ps, off = scores(p, 0)
                    kcs = list(range(4 * j)) + list(range(nch - 1, 4 * j - 1, -1))
                    for ki, kc in enumerate(kcs):
                        if off >= 256:
                            # split per head: skips the unused [512, 512+off)
                            # region and frees the scores PSUM sooner
                            nc.scalar.activation(
                                P[:, 1024 * kc + off : 1024 * kc + 512],
                                ps[:, off:512],
                                EXPF,
                                scale=0.125,
                            )
                            nc.scalar.activation(
                                P[:, 1024 * kc + 512 + off : 1024 * (kc + 1)],
                                ps[:, 512 + off : 1024],
                                EXPF,
                                scale=0.125,
                            )
                        else:
                            nc.scalar.activation(
                                P[:, 1024 * kc + off : 1024 * (kc + 1)],
                                ps[:, off:1024],
                                EXPF,
                                scale=0.125,
                            )
                        if kc >= 4 * j:  # diagonal band: mask 128x128 blocks
                            for h in range(2):
                                sl = slice(
                                    1024 * kc + 512 * h + off,
                                    1024 * kc + 512 * h + off + 128,
                                )
                                nc.gpsimd.tensor_mul(P[:, sl], P[:, sl], maskt[:])
                        # next chunk's scores keep PE busy while ACT runs exp
                        if ki + 1 < nch:
                            for u in pre_units.pop((p, kcs[ki + 1]), ()):
                                u()
                            ps_n, off_n = scores(p, kcs[ki + 1])
                        step += 1
                        while fi < len(fillers) and done_w * steps < total_w * step:
                            done_w += fillers[fi][0]
                            fillers[fi][1]()
                            fi += 1
                        nc.tensor.matmul(
                            av0[:, off:QB],
                            vp[kc][p][:, 0:65],
                            P[:, 1024 * kc + off : 1024 * kc + 512],
                            start=(ki == 0),
                            stop=(ki == nch - 1),
                        )
                        nc.tensor.matmul(
                            av1[:, off:QB],
                            vp[kc][p][:, 65:193],
                            P[:, 1024 * kc + 512 + off : 1024 * (kc + 1)],
                            start=(ki == 0),
                            stop=(ki == nch - 1),
                        )
                        if ki + 1 < nch:
                            ps, off = ps_n, off_n
                    # softmax denominators: av0 row 64 = sum(P_h0),
                    # av1 row 0 = sum(P_h1); av1 values live on rows 64..127
                    rcb = rcbs[rci[0]]
                    rci[0] ^= 1
                    with nc.allow_low_precision("softmax denom recip in fp16"):
                        nc.vector.reciprocal(rcb[64:65, :], av0[64:65, :])
                        nc.vector.reciprocal(rcb[0:1, :], av1[0:1, :])
                    if p == 0:
                        # prefetch the next pair's first scores so PE has
                        # work while the reciprocal lands
                        for u in pre_units.pop((1, 0), ()):
                            u()
                        prefetched[0] = scores(1, 0)
                    bc = psA.tile([128, QB], F32, tag="psA", name=f"bc{j}{p}")
                    nc.tensor.matmul(bc[:], selt[:], rcb[:], start=True, stop=True)
                    bcs = wpool.tile([128, QB], F32, tag="bcs", bufs=2, name="bcs")
                    nc.vector.tensor_copy(bcs[:], bc[:])
                    nc.vector.tensor_mul(
                        ao[p][0:64, QB * j : QB * (j + 1)], av0[0:64, :], bcs[0:64, :]
                    )
                    nc.vector.tensor_mul(
                        ao[p][64:128, QB * j : QB * (j + 1)],
                        av1[64:128, :],
                        bcs[64:128, :],
                    )
                for w, u in fillers[fi:]:
                    u()

            # ---- main schedule: pair-0 projections first so attention(0)
            # can start after two projection calls ----
            for pp, wt, ot in ((0, wq, qt), (0, wk, kt), (1, wq, qt), (1, wk, kt)):
                qk_proj(pp, wt, ot, 0)
            for j in range(NQB):
                fillers = []
                if j + 1 < NQB:
                    jq = j + 1
                    for pp, wt, ot in ((0, wq, qt), (0, wk, kt), (1, wq, qt), (1, wk, kt)):
                        fillers.append(
                            (4096, lambda pp=pp, wt=wt, ot=ot, jq=jq: qk_proj(pp, wt, ot, jq))
                        )
                # out-proj of block 0 fills attention(1); blocks 1 and 2
                # both fill attention(3), which is the ACT-bound stretch
                op_blocks = {1: [0], 3: [1, 2]}.get(j, [])
                for jb in op_blocks:
                    for et in range(8):
                        fillers.append((1024, lambda jb=jb, et=et: out_proj_unit(jb, et)))
                pre_units = {}
                for tt in range(4 * j, 4 * j + 4):
                    pos = 4 * j + (4 * j + 3 - tt)  # position of chunk tt
                    kcs_full = list(range(4 * j)) + list(
                        range(4 * j + 3, 4 * j - 1, -1)
                    )
                    prev_kc = kcs_full[max(0, pos - 1)]
                    pre_units.setdefault((0, prev_kc if pos > 0 else kcs_full[0]), []).append(
                        lambda tt=tt: v_proj(tt)
                    )
                attention(j, fillers, pre_units)
            for et in range(8):
                out_proj_unit(NQB - 1, et, tail=True)
    split_multi_waits(nc)
    return nc


_NC = None


def _get_nc():
    global _NC
    if _NC is None:
        _NC = build_nc()
    return _NC


def kernel(x, W_q, W_k, W_v, W_o):
    x = np.asarray(x, dtype=np.float32)
    W_q = np.asarray(W_q, dtype=np.float32)
    W_k = np.asarray(W_k, dtype=np.float32)
    W_v = np.asarray(W_v, dtype=np.float32)
    W_o = np.asarray(W_o, dtype=np.float32)

    def pack8(a):  # [1024, 256] -> [128, 8*256], chunk kc at cols 256*kc
        return np.ascontiguousarray(
            a.reshape(8, 128, 256).transpose(1, 0, 2).reshape(128, 2048)
        ).astype(np.float16)

    def pack2(a):  # [256, 1024] -> [128, 2*1024], chunk p at cols 1024*p
        return np.ascontiguousarray(
            a.reshape(2, 128, 1024).transpose(1, 0, 2).reshape(128, 2048)
        ).astype(np.float16)

    tmask = np.triu(np.ones((128, 128), dtype=np.float16))
    sel65 = np.zeros((65, 128), dtype=np.float16)
    sel65[64, 0:64] = 1.0
    sel65[0, 64:128] = 1.0
    xTb = [np.ascontiguousarray(x[b].T).astype(np.float16) for b in range(B)]
    in_maps = []
    for c in range(NCORES):
        b, g = c // 4, c % 4
        hs = 256 * g
        in_maps.append(
            {
                "xT": xTb[b],
                "wqT": pack8(W_q[hs : hs + 256, :].T),
                "wkT": pack8(W_k[hs : hs + 256, :].T),
                "wvT": pack8(W_v[hs : hs + 256, :].T),
                "woT": pack2(W_o[:, hs : hs + 256].T),
                "tmask": tmask,
                "sel65": sel65,
            }
        )
    res = run_bass_kernel_spmd(_get_nc(), in_maps, core_ids=list(range(NCORES)))
    out = np.empty((B, T, D), dtype=np.float32)
    for b in range(B):
        acc = res.results[4 * b]["yT"].astype(np.float32)
        for g in range(1, 4):
            acc = acc + res.results[4 * b + g]["yT"]
        out[b] = acc.T
    return out


# revision 54
# speedup vs baseline: 1.0359x; 1.0164x over previous
"""Causal self-attention Trainium2 kernel (B=2, T=2048, D=1024, H=16).

Sharding: 8 cores = 2 batch groups x 4 head groups; each core computes
batch b = c//4, heads 4*(c%4)..4*(c%4)+3 (256 QKV dims), and a partial
output projection y_cT = W_o[:, slice] @ attnout (contribution summed on
host across the 4 cores of each batch group).

All on-device compute in fp16 operands with fp32 PSUM accumulation.
Everything is kept "transposed" ([dim, seq]) so no on-device transposes
are needed:
  QT/KT = W @ xT               [256, 2048]
  V     = x @ WvT              [2048, 256]   (seq on partitions)
  ST[k,q] = sum_d K[k,d]Q[q,d] (k on partitions, q streaming)
  causal mask applied additively on PSUM (Pool engine) before exp
  P = exp(ST/8)  (ACT engine)
  avT[d,q] = sum_k [V|1][k,d] P[k,q]  -> row of ones gives softmax denom
  attnout[d,q] = avT * (1/denom); denom reciprocal broadcast across
  partitions via Pool partition_broadcast
  yT[e,q] = WoT.T @ attnout  (partial over this core's 256 dims)

Schedule: DMAs are split/prioritized so the first projection starts ~1us
in; V projections and the previous block's output projection are deferred
into later (ACT-bound) attention blocks as PE filler work.
"""

import numpy as np

import concourse.bass as bass
import concourse.mybir as mybir
from concourse.tile import TileContext
from concourse.vector_clock import ScopedClock
from concourse.bass_utils import run_bass_kernel_spmd

B, T, D = 2, 2048, 1024
H, DK = 16, 64
NCORES = 8
HPC = 4            # heads per core
QB = 512           # q block size
NQB = T // QB      # 4
NKC = T // 128     # 16 k-chunks
F16 = mybir.dt.float16
F32 = mybir.dt.float32
EXPF = mybir.ActivationFunctionType.Exp
MASK_NEG = -30000.0


class TC(TileContext):
    """This container's walrus only accepts one sync-wait per TPB_CTRL
    instruction; split the tile tail-drain waits into one nop each."""

    def _drain_and_barrier(self, tick_clock, wait_clock):
        carrier = self.nc.sync.nop(nofuse=True)
        wait_clock.add_sem_waits(
            carrier.ins, ScopedClock({None: tick_clock.global_clock})
        )
        si = carrier.ins.sync_info
        if si is not None and len(si.on_wait) > 1:
            waits = list(si.on_wait)
            carrier.ins.sync_info = mybir.SyncInfo(
                on_wait=[waits[0]], on_update=list(si.on_update)
            )
            for w in waits[1:]:
                nop = self.nc.sync.nop(nofuse=True)
                nop.ins.sync_info = mybir.SyncInfo(on_wait=[w], on_update=[])
        self.nc.sync.drain()
        self.nc.all_engine_barrier()
        assert self.sems is not None
        popped = self.nc._tile_sem_poison_stack.pop()
        assert popped is self._sem_poison
        self.nc.clear_and_free_semaphores(list(self.sems.allocated().values()))
        self.nc.all_engine_barrier()


def split_multi_waits(nc):
    """This walrus build accepts only one sync-wait per instruction; hoist
    extra waits onto single-wait NoOps inserted just before the instruction
    on the same engine."""
    for fn in nc.m.functions:
        for bb in fn.blocks:
            out = []
            for ins in bb.instructions:
                si = getattr(ins, "sync_info", None)
                is_isa = "ISA" in type(ins).__name__ or "PartitionBroadcast" in type(ins).__name__
                keep = 0 if is_isa else 1
                if si is not None and len(si.on_wait) > keep:
                    waits = list(si.on_wait)
                    keep_waits = waits[len(waits) - keep :] if keep else []
                    for i, w in enumerate(waits[: len(waits) - keep]):
                        out.append(
                            mybir.InstNoOp(
                                name=f"{ins.name}_w{i}",
                                engine=ins.engine,
                                sync_info=mybir.SyncInfo(on_wait=[w], on_update=[]),
                                bass_nofuse=True,
                            )
                        )
                    ins.sync_info = mybir.SyncInfo(
                        on_wait=keep_waits, on_update=list(si.on_update)
                    )
                out.append(ins)
            bb.instructions = out


def build_nc():
    nc = bass.Bass("TRN2", target_bir_lowering=False, debug=False)
    xT = nc.dram_tensor("xT", [D, T], F16, kind="ExternalInput")
    wqT = nc.dram_tensor("wqT", [128, 2048], F16, kind="ExternalInput")
    wkT = nc.dram_tensor("wkT", [128, 2048], F16, kind="ExternalInput")
    wvT = nc.dram_tensor("wvT", [128, 2048], F16, kind="ExternalInput")
    woT = nc.dram_tensor("woT", [128, 2048], F16, kind="ExternalInput")
    tmask = nc.dram_tensor("tmask", [128, 128], F16, kind="ExternalInput")
    sel65 = nc.dram_tensor("sel65", [65, 128], F16, kind="ExternalInput")
    yT = nc.dram_tensor("yT", [D, T], F16, kind="ExternalOutput")

    with TC(nc) as tc:
        with (
            tc.tile_pool(name="const", bufs=1) as cpool,
            tc.tile_pool(name="work", bufs=2) as wpool,
            tc.tile_pool(name="psA", bufs=2, space="PSUM") as psA,
            tc.tile_pool(name="psS", bufs=2, space="PSUM") as psS,
            tc.tile_pool(name="psV", bufs=1, space="PSUM") as psV,
        ):
            # ---- tiles ----
            xt = [cpool.tile([128, T], F16, tag=f"xt{kc}", name=f"xt{kc}") for kc in range(8)]
            wq = cpool.tile([128, 2048], F16, tag="wq", name="wq")
            wk = cpool.tile([128, 2048], F16, tag="wk", name="wk")
            wv = cpool.tile([128, 2048], F16, tag="wv", name="wv")
            wo = cpool.tile([128, 2048], F16, tag="wo", name="wo")
            maskt = cpool.tile([128, 128], F16, tag="mask")
            selt = cpool.tile([65, 128], F16, tag="sel")

            # warm the ACT exp table while DMAs run
            warm = cpool.tile([1, 8], F32, tag="warm")
            nc.vector.memset(warm[:], 0.0)
            nc.scalar.activation(warm[:], warm[:], EXPF, scale=1.0)

            # ---- input DMAs split across the two HWDGE queues (SP and
            # ACT): urgent first-projection inputs on SP, the rest on ACT ----
            def xf(kc):
                nc.sync.dma_start(
                    xt[kc][:, 0:QB], xT[128 * kc : 128 * (kc + 1), 0:QB]
                )

            nc.sync.dma_start(wq[:, 0:1024], wqT[:, 0:1024])
            xf(0)
            xf(1)
            nc.sync.dma_start(wq[:, 1024:2048], wqT[:, 1024:2048])
            xf(2)
            nc.sync.dma_start(wk[:], wkT[:, :])
            for kc in range(3, 8):
                xf(kc)
            nc.sync.dma_start(maskt[:], tmask[:, :])
            nc.sync.dma_start(selt[:], sel65[:, :])
            nc.sync.dma_start(wv[:], wvT[:, :])
            for kc in range(8):
                nc.sync.dma_start(
                    xt[kc][:, QB:T], xT[128 * kc : 128 * (kc + 1), QB:T]
                )
            nc.sync.dma_start(wo[:], woT[:, :])

            qt = [cpool.tile([128, T], F16, tag=f"qt{p}", name=f"qt{p}") for p in range(2)]
            kt = [cpool.tile([128, T], F16, tag=f"kt{p}", name=f"kt{p}") for p in range(2)]
            ao = [cpool.tile([128, T], F16, tag=f"ao{p}", name=f"ao{p}") for p in range(2)]
            vp = [
                [cpool.tile([128, 193], F16, tag=f"vp{tt}_{p}", name=f"vp{tt}_{p}") for p in range(2)]
                for tt in range(NKC)
            ]

            # ---- Q, K projections: out[p][:, jq] = W.T @ xT ----
            def qk_proj(p, wt, out_t, jq):
                ps = psA.tile([128, QB], F32, tag="psA", name=f"psqk{p}{jq}")
                for kc in range(8):
                    nc.tensor.matmul(
                        ps[:],
                        wt[:, 256 * kc + 128 * p : 256 * kc + 128 * (p + 1)],
                        xt[kc][:, QB * jq : QB * (jq + 1)],
                        start=(kc == 0),
                        stop=(kc == 7),
                    )
                nc.scalar.copy(out_t[p][:, QB * jq : QB * (jq + 1)], ps[:])

            def v_proj(tt):
                ps = psA.tile([128, QB], F32, tag="psA", name=f"psv{tt}")
                for kc in range(8):
                    nc.tensor.matmul(
                        ps[:, 0:256],
                        xt[kc][:, 128 * tt : 128 * (tt + 1)],
                        wv[:, 256 * kc : 256 * (kc + 1)],
                        start=(kc == 0),
                        stop=(kc == 7),
                    )
                for p in range(2):
                    v = vp[tt][p]
                    nc.gpsimd.memset(v[:, 64:66], 1.0)
                    nc.gpsimd.memset(v[:, 66:129], 0.0)
                    nc.vector.tensor_copy(v[:, 0:64], ps[:, 128 * p : 128 * p + 64])
                    nc.vector.tensor_copy(v[:, 129:193], ps[:, 128 * p + 64 : 128 * p + 128])

            # ---- output projection for one (q block, dim chunk) ----
            def out_proj_unit(j, et, tail=False):
                ps = psA.tile([128, QB], F32, tag="psA", name=f"pso{j}{et}")
                for p in range(2):
                    nc.tensor.matmul(
                        ps[:],
                        wo[:, 1024 * p + 128 * et : 1024 * p + 128 * (et + 1)],
                        ao[p][:, QB * j : QB * (j + 1)],
                        start=(p == 0),
                        stop=(p == 1),
                    )
                ysb = wpool.tile([128, QB], F16, tag="ysb", bufs=8, name="ysb")
                if tail and et % 2:
                    nc.scalar.copy(ysb[:], ps[:])
                else:
                    nc.vector.tensor_copy(ysb[:], ps[:])
                ring = nc.scalar if (tail and et % 2) else nc.sync
                ring.dma_start(
                    yT[128 * et : 128 * (et + 1), QB * j : QB * (j + 1)], ysb[:]
                )

            # reciprocal staging tiles: rows 1..63 must never hold NaN (the
            # sel matmul multiplies them by zero); zero them once up front
            rcbs = [
                cpool.tile([65, QB], F16, tag=f"rcb{i}", name=f"rcb{i}")
                for i in range(2)
            ]
            for t in rcbs:
                nc.gpsimd.memset(t[:], 0.0)
            rci = [0]

            # ---- attention over one q block; fillers are (weight, closure)
            # PE work consumed pro-rata across chunk steps; pre_units are
            # placed before the scores of a specific (p, kc) chunk ----
            def attention(j, fillers, pre_units):
                total_w = sum(w for w, _ in fillers) or 1
                done_w = 0
                fi = 0
                nch = 4 * j + 4
                steps = 2 * nch
                step = 0

                def scores(p, kc):
                    off = max(0, 128 * (kc - 4 * j))
                    ps = psS.tile([128, 1024], F32, tag="psS", name="psS")
                    for h in range(2):
                        nc.tensor.matmul(
                            ps[:, 512 * h + off : 512 * (h + 1)],
                            kt[p][64 * h : 64 * (h + 1), 128 * kc : 128 * (kc + 1)],
                            qt[p][64 * h : 64 * (h + 1), QB * j + off : QB * (j + 1)],
                            start=True,
                            stop=True,
                        )
                    return ps, off

                prefetched = [None]
                for p in range(2):
                    P = wpool.tile([128, 1024 * NKC], F16, tag="P", bufs=2, name="P")
                    av0 = psV.tile([65, QB], F32, tag="av0", name="av0")
                    av1 = psV.tile([128, QB], F32, tag="av1", name="av1")
                    if prefetched[0] is not None:
                        ps, off = prefetched[0]
                        prefetched[0] = None
                    else:
                        for u in pre_units.pop((p, 0), ()):
                            u()
                        ps, off = scores(p, 0)
                    for kc in range(nch):
                        if off >= 256:
                            # split per head: skips the unused [512, 512+off)
                            # region and frees the scores PSUM sooner
                            nc.scalar.activation(
                                P[:, 1024 * kc + off : 1024 * kc + 512],
                                ps[:, off:512],
                                EXPF,
                                scale=0.125,
                            )
                            nc.scalar.activation(
                                P[:, 1024 * kc + 512 + off : 1024 * (kc + 1)],
                                ps[:, 512 + off : 1024],
                                EXPF,
                                scale=0.125,
                            )
                        else:
                            nc.scalar.activation(
                                P[:, 1024 * kc + off : 1024 * (kc + 1)],
                                ps[:, off:1024],
                                EXPF,
                                scale=0.125,
                            )
                        if kc >= 4 * j:  # diagonal band: mask 128x128 blocks
                            for h in range(2):
                                sl = slice(
                                    1024 * kc + 512 * h + off,
                                    1024 * kc + 512 * h + off + 128,
                                )
                                nc.gpsimd.tensor_mul(P[:, sl], P[:, sl], maskt[:])
                        # next chunk's scores keep PE busy while ACT runs exp
                        if kc + 1 < nch:
                            for u in pre_units.pop((p, kc + 1), ()):
                                u()
                            ps_n, off_n = scores(p, kc + 1)
                        step += 1
                        while fi < len(fillers) and done_w * steps < total_w * step:
                            done_w += fillers[fi][0]
                            fillers[fi][1]()
                            fi += 1
                        nc.tensor.matmul(
                            av0[:, off:QB],
                            vp[kc][p][:, 0:65],
                            P[:, 1024 * kc + off : 1024 * kc + 512],
                            start=(kc == 0),
                            stop=(kc == nch - 1),
                        )
                        nc.tensor.matmul(
                            av1[:, off:QB],
                            vp[kc][p][:, 65:193],
                            P[:, 1024 * kc + 512 + off : 1024 * (kc + 1)],
                            start=(kc == 0),
                            stop=(kc == nch - 1),
                        )
                        if kc + 1 < nch:
                            ps, off = ps_n, off_n
                    # softmax denominators: av0 row 64 = sum(P_h0),
                    # av1 row 0 = sum(P_h1); av1 values live on rows 64..127
                    rcb = rcbs[rci[0]]
                    rci[0] ^= 1
                    with nc.allow_low_precision("softmax denom recip in fp16"):
                        nc.vector.reciprocal(rcb[64:65, :], av0[64:65, :])
                        nc.vector.reciprocal(rcb[0:1, :], av1[0:1, :])
                    if p == 0:
                        # prefetch the next pair's first scores so PE has
                        # work while the reciprocal lands
                        for u in pre_units.pop((1, 0), ()):
                            u()
                        prefetched[0] = scores(1, 0)
                    bc = psA.tile([128, QB], F32, tag="psA", name=f"bc{j}{p}")
                    nc.tensor.matmul(bc[:], selt[:], rcb[:], start=True, stop=True)
                    bcs = wpool.tile([128, QB], F32, tag="bcs", bufs=2, name="bcs")
                    nc.vector.tensor_copy(bcs[:], bc[:])
                    nc.vector.tensor_mul(
                        ao[p][0:64, QB * j : QB * (j + 1)], av0[0:64, :], bcs[0:64, :]
                    )
                    nc.vector.tensor_mul(
                        ao[p][64:128, QB * j : QB * (j + 1)],
                        av1[64:128, :],
                        bcs[64:128, :],
                    )
                for w, u in fillers[fi:]:
                    u()

            # ---- main schedule: pair-0 projections first so attention(0)
            # can start after two projection calls ----
            for pp, wt, ot in ((0, wq, qt), (0, wk, kt), (1, wq, qt), (1, wk, kt)):
                qk_proj(pp, wt, ot, 0)
            for j in range(NQB):
                fillers = []
                if j + 1 < NQB:
                    jq = j + 1
                    for pp, wt, ot in ((0, wq, qt), (0, wk, kt), (1, wq, qt), (1, wk, kt)):
                        fillers.append(
                            (4096, lambda pp=pp, wt=wt, ot=ot, jq=jq: qk_proj(pp, wt, ot, jq))
                        )
                # out-proj of block 0 fills attention(1); blocks 1 and 2
                # both fill attention(3), which is the ACT-bound stretch
                op_blocks = {1: [0], 3: [1, 2]}.get(j, [])
                for jb in op_blocks:
                    for et in range(8):
                        fillers.append((1024, lambda jb=jb, et=et: out_proj_unit(jb, et)))
                pre_units = {}
                for tt in range(4 * j, 4 * j + 4):
                    pre_units.setdefault((0, max(0, tt - 1)), []).append(
                        lambda tt=tt: v_proj(tt)
                    )
                attention(j, fillers, pre_units)
            for et in range(8):
                out_proj_unit(NQB - 1, et, tail=True)
    split_multi_waits(nc)
    return nc


_NC = None


def _get_nc():
    global _NC
    if _NC is None:
        _NC = build_nc()
    return _NC


def kernel(x, W_q, W_k, W_v, W_o):
    x = np.asarray(x, dtype=np.float32)
    W_q = np.asarray(W_q, dtype=np.float32)
    W_k = np.asarray(W_k, dtype=np.float32)
    W_v = np.asarray(W_v, dtype=np.float32)
    W_o = np.asarray(W_o, dtype=np.float32)

    def pack8(a):  # [1024, 256] -> [128, 8*256], chunk kc at cols 256*kc
        return np.ascontiguousarray(
            a.reshape(8, 128, 256).transpose(1, 0, 2).reshape(128, 2048)
        ).astype(np.float16)

    def pack2(a):  # [256, 1024] -> [128, 2*1024], chunk p at cols 1024*p
        return np.ascontiguousarray(
            a.reshape(2, 128, 1024).transpose(1, 0, 2).reshape(128, 2048)
        ).astype(np.float16)

    tmask = np.triu(np.ones((128, 128), dtype=np.float16))
    sel65 = np.zeros((65, 128), dtype=np.float16)
    sel65[64, 0:64] = 1.0
    sel65[0, 64:128] = 1.0
    xTb = [np.ascontiguousarray(x[b].T).astype(np.float16) for b in range(B)]
    in_maps = []
    for c in range(NCORES):
        b, g = c // 4, c % 4
        hs = 256 * g
        in_maps.append(
            {
                "xT": xTb[b],
                "wqT": pack8(W_q[hs : hs + 256, :].T),
                "wkT": pack8(W_k[hs : hs + 256, :].T),
                "wvT": pack8(W_v[hs : hs + 256, :].T),
                "woT": pack2(W_o[:, hs : hs + 256].T),
                "tmask": tmask,
                "sel65": sel65,
            }
        )
    res = run_bass_kernel_spmd(_get_nc(), in_maps, core_ids=list(range(NCORES)))
    out = np.empty((B, T, D), dtype=np.float32)
    for b in range(B):
        acc = res.results[4 * b]["yT"].astype(np.float32)
        for g in range(1, 4):
            acc = acc + res.results[4 * b + g]["yT"]
        out[b] = acc.T
    return out


# revision 67
# speedup vs baseline: 1.0642x; 1.0273x over previous
"""Causal self-attention Trainium2 kernel (B=2, T=2048, D=1024, H=16).

Sharding: 8 cores = 2 batch groups x 4 head groups; each core computes
batch b = c//4, heads 4*(c%4)..4*(c%4)+3 (256 QKV dims), and a partial
output projection y_cT = W_o[:, slice] @ attnout (contribution summed on
host across the 4 cores of each batch group).

All on-device compute in fp16 operands with fp32 PSUM accumulation.
Everything is kept "transposed" ([dim, seq]) so no on-device transposes
are needed:
  QT/KT = W @ xT               [256, 2048]
  V     = x @ WvT              [2048, 256]   (seq on partitions)
  ST[k,q] = sum_d K[k,d]Q[q,d] (k on partitions, q streaming)
  causal mask applied additively on PSUM (Pool engine) before exp
  P = exp(ST/8)  (ACT engine)
  avT[d,q] = sum_k [V|1][k,d] P[k,q]  -> row of ones gives softmax denom
  attnout[d,q] = avT * (1/denom); denom reciprocal broadcast across
  partitions via Pool partition_broadcast
  yT[e,q] = WoT.T @ attnout  (partial over this core's 256 dims)

Schedule: DMAs are split/prioritized so the first projection starts ~1us
in; V projections and the previous block's output projection are deferred
into later (ACT-bound) attention blocks as PE filler work.
"""

import numpy as np

import concourse.bass as bass
import concourse.mybir as mybir
from concourse.tile import TileContext
from concourse.vector_clock import ScopedClock
from concourse.bass_utils import run_bass_kernel_spmd

B, T, D = 2, 2048, 1024
H, DK = 16, 64
NCORES = 8
HPC = 4            # heads per core
QB = 512           # q block size
NQB = T // QB      # 4
NKC = T // 128     # 16 k-chunks
F16 = mybir.dt.float16
F32 = mybir.dt.float32
EXPF = mybir.ActivationFunctionType.Exp
MASK_NEG = -30000.0


class TC(TileContext):
    """This container's walrus only accepts one sync-wait per TPB_CTRL
    instruction; split the tile tail-drain waits into one nop each."""

    def _drain_and_barrier(self, tick_clock, wait_clock):
        carrier = self.nc.sync.nop(nofuse=True)
        wait_clock.add_sem_waits(
            carrier.ins, ScopedClock({None: tick_clock.global_clock})
        )
        si = carrier.ins.sync_info
        if si is not None and len(si.on_wait) > 1:
            waits = list(si.on_wait)
            carrier.ins.sync_info = mybir.SyncInfo(
                on_wait=[waits[0]], on_update=list(si.on_update)
            )
            for w in waits[1:]:
                nop = self.nc.sync.nop(nofuse=True)
                nop.ins.sync_info = mybir.SyncInfo(on_wait=[w], on_update=[])
        self.nc.sync.drain()
        self.nc.all_engine_barrier()
        assert self.sems is not None
        popped = self.nc._tile_sem_poison_stack.pop()
        assert popped is self._sem_poison
        self.nc.clear_and_free_semaphores(list(self.sems.allocated().values()))
        self.nc.all_engine_barrier()


def split_multi_waits(nc):
    """This walrus build accepts only one sync-wait per instruction; hoist
    extra waits onto single-wait NoOps inserted just before the instruction
    on the same engine."""
    for fn in nc.m.functions:
        for bb in fn.blocks:
            out = []
            for ins in bb.instructions:
                si = getattr(ins, "sync_info", None)
                is_isa = "ISA" in type(ins).__name__ or "PartitionBroadcast" in type(ins).__name__
                keep = 0 if is_isa else 1
                if si is not None and len(si.on_wait) > keep:
                    waits = list(si.on_wait)
                    keep_waits = waits[len(waits) - keep :] if keep else []
                    for i, w in enumerate(waits[: len(waits) - keep]):
                        out.append(
                            mybir.InstNoOp(
                                name=f"{ins.name}_w{i}",
                                engine=ins.engine,
                                sync_info=mybir.SyncInfo(on_wait=[w], on_update=[]),
                                bass_nofuse=True,
                            )
                        )
                    ins.sync_info = mybir.SyncInfo(
                        on_wait=keep_waits, on_update=list(si.on_update)
                    )
                out.append(ins)
            bb.instructions = out


def build_nc():
    nc = bass.Bass("TRN2", target_bir_lowering=False, debug=False)
    xT = nc.dram_tensor("xT", [D, T], F16, kind="ExternalInput")
    wqT = nc.dram_tensor("wqT", [128, 2048], F16, kind="ExternalInput")
    wkT = nc.dram_tensor("wkT", [128, 2048], F16, kind="ExternalInput")
    wvT = nc.dram_tensor("wvT", [128, 2048], F16, kind="ExternalInput")
    woT = nc.dram_tensor("woT", [128, 2048], F16, kind="ExternalInput")
    tmask = nc.dram_tensor("tmask", [128, 128], F16, kind="ExternalInput")
    sel65 = nc.dram_tensor("sel65", [65, 128], F16, kind="ExternalInput")
    yT = nc.dram_tensor("yT", [D, T], F16, kind="ExternalOutput")

    with TC(nc) as tc:
        with (
            tc.tile_pool(name="const", bufs=1) as cpool,
            tc.tile_pool(name="work", bufs=2) as wpool,
            tc.tile_pool(name="psA", bufs=2, space="PSUM") as psA,
            tc.tile_pool(name="psS", bufs=2, space="PSUM") as psS,
            tc.tile_pool(name="psV", bufs=1, space="PSUM") as psV,
        ):
            # ---- tiles ----
            xt = [cpool.tile([128, T], F16, tag=f"xt{kc}", name=f"xt{kc}") for kc in range(8)]
            wq = cpool.tile([128, 2048], F16, tag="wq", name="wq")
            wk = cpool.tile([128, 2048], F16, tag="wk", name="wk")
            wv = cpool.tile([128, 2048], F16, tag="wv", name="wv")
            wo = cpool.tile([128, 2048], F16, tag="wo", name="wo")
            maskt = cpool.tile([128, 128], F16, tag="mask")
            selt = cpool.tile([65, 128], F16, tag="sel")

            # warm the ACT exp table while DMAs run
            warm = cpool.tile([1, 8], F32, tag="warm")
            nc.vector.memset(warm[:], 0.0)
            nc.scalar.activation(warm[:], warm[:], EXPF, scale=1.0)

            # ---- input DMAs split across the two HWDGE queues (SP and
            # ACT): urgent first-projection inputs on SP, the rest on ACT ----
            def xf(kc):
                nc.sync.dma_start(
                    xt[kc][:, 0:QB], xT[128 * kc : 128 * (kc + 1), 0:QB]
                )

            nc.sync.dma_start(wq[:, 0:1024], wqT[:, 0:1024])
            xf(0)
            xf(1)
            nc.sync.dma_start(wq[:, 1024:2048], wqT[:, 1024:2048])
            xf(2)
            nc.sync.dma_start(wk[:], wkT[:, :])
            for kc in range(3, 8):
                xf(kc)
            nc.sync.dma_start(maskt[:], tmask[:, :])
            nc.sync.dma_start(selt[:], sel65[:, :])
            nc.sync.dma_start(wv[:], wvT[:, :])
            for kc in range(8):
                nc.sync.dma_start(
                    xt[kc][:, QB:T], xT[128 * kc : 128 * (kc + 1), QB:T]
                )
            nc.sync.dma_start(wo[:], woT[:, :])

            qt = [cpool.tile([128, T], F16, tag=f"qt{p}", name=f"qt{p}") for p in range(2)]
            kt = [cpool.tile([128, T], F16, tag=f"kt{p}", name=f"kt{p}") for p in range(2)]
            ao = [cpool.tile([128, T], F16, tag=f"ao{p}", name=f"ao{p}") for p in range(2)]
            vp = [
                [cpool.tile([128, 193], F16, tag=f"vp{tt}_{p}", name=f"vp{tt}_{p}") for p in range(2)]
                for tt in range(NKC)
            ]

            # ---- Q, K projections: out[p][:, jq] = W.T @ xT ----
            def qk_proj(p, wt, out_t, jq):
                ps = psA.tile([128, QB], F32, tag="psA", name=f"psqk{p}{jq}")
                for kc in range(8):
                    nc.tensor.matmul(
                        ps[:],
                        wt[:, 256 * kc + 128 * p : 256 * kc + 128 * (p + 1)],
                        xt[kc][:, QB * jq : QB * (jq + 1)],
                        start=(kc == 0),
                        stop=(kc == 7),
                    )
                nc.scalar.copy(out_t[p][:, QB * jq : QB * (jq + 1)], ps[:])

            def v_proj(tt):
                ps = psA.tile([128, QB], F32, tag="psA", name=f"psv{tt}")
                for kc in range(8):
                    nc.tensor.matmul(
                        ps[:, 0:256],
                        xt[kc][:, 128 * tt : 128 * (tt + 1)],
                        wv[:, 256 * kc : 256 * (kc + 1)],
                        start=(kc == 0),
                        stop=(kc == 7),
                    )
                for p in range(2):
                    v = vp[tt][p]
                    nc.gpsimd.memset(v[:, 64:66], 1.0)
                    nc.gpsimd.memset(v[:, 66:129], 0.0)
                    nc.vector.tensor_copy(v[:, 0:64], ps[:, 128 * p : 128 * p + 64])
                    nc.vector.tensor_copy(v[:, 129:193], ps[:, 128 * p + 64 : 128 * p + 128])

            # ---- output projection for one (q block, dim chunk) ----
            def out_proj_unit(j, et, tail=False):
                ps = psA.tile([128, QB], F32, tag="psA", name=f"pso{j}{et}")
                for p in range(2):
                    nc.tensor.matmul(
                        ps[:],
                        wo[:, 1024 * p + 128 * et : 1024 * p + 128 * (et + 1)],
                        ao[p][:, QB * j : QB * (j + 1)],
                        start=(p == 0),
                        stop=(p == 1),
                    )
                ysb = wpool.tile([128, QB], F16, tag="ysb", bufs=8, name="ysb")
                if tail and et % 2:
                    nc.scalar.copy(ysb[:], ps[:])
                else:
                    nc.vector.tensor_copy(ysb[:], ps[:])
                ring = nc.scalar if (tail and et % 2) else nc.sync
                ring.dma_start(
                    yT[128 * et : 128 * (et + 1), QB * j : QB * (j + 1)], ysb[:]
                )

            # reciprocal staging tiles: rows 1..63 must never hold NaN (the
            # sel matmul multiplies them by zero); zero them once up front
            rcbs = [
                cpool.tile([65, QB], F16, tag=f"rcb{i}", name=f"rcb{i}")
                for i in range(2)
            ]
            for t in rcbs:
                nc.gpsimd.memset(t[:], 0.0)
            rci = [0]

            # ---- attention over one q block; fillers are (weight, closure)
            # PE work consumed pro-rata across chunk steps; pre_units are
            # placed before the scores of a specific (p, kc) chunk ----
            def mk_scores(j):
                def scores(p, kc):
                    off = max(0, 128 * (kc - 4 * j))
                    ps = psS.tile([128, 1024], F32, tag="psS", name="psS")
                    for h in range(2):
                        nc.tensor.matmul(
                            ps[:, 512 * h + off : 512 * (h + 1)],
                            kt[p][64 * h : 64 * (h + 1), 128 * kc : 128 * (kc + 1)],
                            qt[p][64 * h : 64 * (h + 1), QB * j + off : QB * (j + 1)],
                            start=True,
                            stop=True,
                        )
                    return ps, off

                return scores

            def attention(j, fillers, pre_units, carry_in=None, next_first=None):
                scores = mk_scores(j)
                total_w = sum(w for w, _ in fillers) or 1
                done_w = 0
                fi = 0
                nch = 4 * j + 4
                steps = 2 * nch
                step = 0
                carry_out = [None]

                prefetched = [carry_in]
                for p in range(2):
                    P = wpool.tile([128, 1024 * NKC], F16, tag="P", bufs=3, name="P")
                    av0 = psV.tile([65, QB], F32, tag="av0", name="av0")
                    av1 = psV.tile([128, QB], F32, tag="av1", name="av1")
                    if prefetched[0] is not None:
                        queue = [prefetched[0]]
                        prefetched[0] = None
                    else:
                        for u in pre_units.pop((p, 0), ()):
                            u()
                        queue = [scores(p, 0)]
                    if nch > 1:
                        for u in pre_units.pop((p, 1), ()):
                            u()
                        queue.append(scores(p, 1))
                    for kc in range(nch):
                        ps, off = queue.pop(0)
                        if off >= 256:
                            # split per head: skips the unused [512, 512+off)
                            # region and frees the scores PSUM sooner
                            nc.scalar.activation(
                                P[:, 1024 * kc + off : 1024 * kc + 512],
                                ps[:, off:512],
                                EXPF,
                                scale=0.125,
                            )
                            nc.scalar.activation(
                                P[:, 1024 * kc + 512 + off : 1024 * (kc + 1)],
                                ps[:, 512 + off : 1024],
                                EXPF,
                                scale=0.125,
                            )
                        else:
                            nc.scalar.activation(
                                P[:, 1024 * kc + off : 1024 * (kc + 1)],
                                ps[:, off:1024],
                                EXPF,
                                scale=0.125,
                            )
                        if kc >= 4 * j:  # diagonal band: mask 128x128 blocks
                            for h in range(2):
                                sl = slice(
                                    1024 * kc + 512 * h + off,
                                    1024 * kc + 512 * h + off + 128,
                                )
                                nc.gpsimd.tensor_mul(P[:, sl], P[:, sl], maskt[:])
                        # next chunk's scores keep PE busy while ACT runs exp
                        if kc + 2 < nch:
                            for u in pre_units.pop((p, kc + 2), ()):
                                u()
                            queue.append(scores(p, kc + 2))
                        step += 1
                        if kc < nch - 1:
                            while fi < len(fillers) and done_w * steps < total_w * step:
                                done_w += fillers[fi][0]
                                fillers[fi][1]()
                                fi += 1
                        nc.tensor.matmul(
                            av0[:, off:QB],
                            vp[kc][p][:, 0:65],
                            P[:, 1024 * kc + off : 1024 * kc + 512],
                            start=(kc == 0),
                            stop=(kc == nch - 1),
                        )
                        nc.tensor.matmul(
                            av1[:, off:QB],
                            vp[kc][p][:, 65:193],
                            P[:, 1024 * kc + 512 + off : 1024 * (kc + 1)],
                            start=(kc == 0),
                            stop=(kc == nch - 1),
                        )
                    # softmax denominators: av0 row 64 = sum(P_h0),
                    # av1 row 0 = sum(P_h1); av1 values live on rows 64..127
                    rcb = rcbs[rci[0]]
                    rci[0] ^= 1
                    with nc.allow_low_precision("softmax denom recip in fp16"):
                        nc.vector.reciprocal(rcb[64:65, :], av0[64:65, :])
                        nc.vector.reciprocal(rcb[0:1, :], av1[0:1, :])
                    while fi < len(fillers) and done_w * steps < total_w * step:
                        done_w += fillers[fi][0]
                        fillers[fi][1]()
                        fi += 1
                    if p == 0:
                        # prefetch the next pair's first scores so PE has
                        # work while the reciprocal lands
                        for u in pre_units.pop((1, 0), ()):
                            u()
                        prefetched[0] = scores(1, 0)
                    bc = psA.tile([128, QB], F32, tag="psA", name=f"bc{j}{p}")
                    nc.tensor.matmul(bc[:], selt[:], rcb[:], start=True, stop=True)
                    bcs = wpool.tile([128, QB], F32, tag="bcs", bufs=2, name="bcs")
                    nc.vector.tensor_copy(bcs[:], bc[:])
                    nc.vector.tensor_mul(
                        ao[p][0:64, QB * j : QB * (j + 1)], av0[0:64, :], bcs[0:64, :]
                    )
                    nc.vector.tensor_mul(
                        ao[p][64:128, QB * j : QB * (j + 1)],
                        av1[64:128, :],
                        bcs[64:128, :],
                    )
                for w, u in fillers[fi:]:
                    u()
                if next_first is not None:
                    carry_out[0] = next_first()
                return carry_out[0]

            # ---- main schedule: pair-0 projections first so attention(0)
            # can start after two projection calls ----
            for pp, wt, ot in ((0, wq, qt), (0, wk, kt), (1, wq, qt), (1, wk, kt)):
                qk_proj(pp, wt, ot, 0)
            carry = None
            first_scores = {jj: (lambda jj=jj: mk_scores(jj)(0, 0)) for jj in range(NQB)}
            for j in range(NQB):
                fillers = []
                if j + 1 < NQB:
                    jq = j + 1
                    for pp, wt, ot in ((0, wq, qt), (0, wk, kt), (1, wq, qt), (1, wk, kt)):
                        fillers.append(
                            (4096, lambda pp=pp, wt=wt, ot=ot, jq=jq: qk_proj(pp, wt, ot, jq))
                        )
                # out-proj of block 0 fills attention(1); blocks 1 and 2
                # both fill attention(3), which is the ACT-bound stretch
                op_blocks = {1: [0], 3: [1, 2]}.get(j, [])
                for jb in op_blocks:
                    for et in range(8):
                        fillers.append((1024, lambda jb=jb, et=et: out_proj_unit(jb, et)))
                pre_units = {}
                for tt in range(4 * j, 4 * j + 4):
                    pre_units.setdefault((0, max(0, tt - 1)), []).append(
                        lambda tt=tt: v_proj(tt)
                    )
                nf = (
                    (lambda jn=j + 1: first_scores[jn]())
                    if j + 1 < NQB
                    else None
                )
                carry = attention(j, fillers, pre_units, carry, nf)
            for et in range(8):
                out_proj_unit(NQB - 1, et, tail=True)
    split_multi_waits(nc)
    return nc


_NC = None


def _get_nc():
    global _NC
    if _NC is None:
        _NC = build_nc()
    return _NC


def kernel(x, W_q, W_k, W_v, W_o):
    x = np.asarray(x, dtype=np.float32)
    W_q = np.asarray(W_q, dtype=np.float32)
    W_k = np.asarray(W_k, dtype=np.float32)
    W_v = np.asarray(W_v, dtype=np.float32)
    W_o = np.asarray(W_o, dtype=np.float32)

    def pack8(a):  # [1024, 256] -> [128, 8*256], chunk kc at cols 256*kc
        return np.ascontiguousarray(
            a.reshape(8, 128, 256).transpose(1, 0, 2).reshape(128, 2048)
        ).astype(np.float16)

    def pack2(a):  # [256, 1024] -> [128, 2*1024], chunk p at cols 1024*p
        return np.ascontiguousarray(
            a.reshape(2, 128, 1024).transpose(1, 0, 2).reshape(128, 2048)
        ).astype(np.float16)

    tmask = np.triu(np.ones((128, 128), dtype=np.float16))
    sel65 = np.zeros((65, 128), dtype=np.float16)
    sel65[64, 0:64] = 1.0
    sel65[0, 64:128] = 1.0
    xTb = [np.ascontiguousarray(x[b].T).astype(np.float16) for b in range(B)]
    in_maps = []
    for c in range(NCORES):
        b, g = c // 4, c % 4
        hs = 256 * g
        in_maps.append(
            {
                "xT": xTb[b],
                "wqT": pack8(W_q[hs : hs + 256, :].T),
                "wkT": pack8(W_k[hs : hs + 256, :].T),
                "wvT": pack8(W_v[hs : hs + 256, :].T),
                "woT": pack2(W_o[:, hs : hs + 256].T),
                "tmask": tmask,
                "sel65": sel65,
            }
        )
    res = run_bass_kernel_spmd(_get_nc(), in_maps, core_ids=list(range(NCORES)))
    out = np.empty((B, T, D), dtype=np.float32)
    for b in range(B):
        acc = res.results[4 * b]["yT"].astype(np.float32)
        for g in range(1, 4):
            acc = acc + res.results[4 * b + g]["yT"]
        out[b] = acc.T
    return out


# revision 73
# speedup vs baseline: 1.0689x; 1.0045x over previous
"""Causal self-attention Trainium2 kernel (B=2, T=2048, D=1024, H=16).

Sharding: 8 cores = 2 batch groups x 4 head groups; each core computes
batch b = c//4, heads 4*(c%4)..4*(c%4)+3 (256 QKV dims), and a partial
output projection y_cT = W_o[:, slice] @ attnout (contribution summed on
host across the 4 cores of each batch group).

All on-device compute in fp16 operands with fp32 PSUM accumulation.
Everything is kept "transposed" ([dim, seq]) so no on-device transposes
are needed:
  QT/KT = W @ xT               [256, 2048]
  V     = x @ WvT              [2048, 256]   (seq on partitions)
  ST[k,q] = sum_d K[k,d]Q[q,d] (k on partitions, q streaming)
  P = exp(ST/8); causal mask applied multiplicatively on the diagonal
  128x128 blocks (Pool engine, SBUF only - Pool cannot touch PSUM)
  avT[d,q] = sum_k [V|1][k,d] P[k,q]  -> row of ones gives softmax denom
  attnout[d,q] = avT * (1/denom); the reciprocal is broadcast across
  partitions with a small selector matmul (PE)
  yT[e,q] = WoT.T @ attnout  (partial over this core's 256 dims)

Schedule highlights (all found against the TimelineSim cost model):
  - input DMAs are split and priority-ordered (weights host-packed to
    [128, 2048] so each loads in one descriptor); the first projection
    starts ~4us in
  - V projections, the next block's Q/K projections and earlier blocks'
    output projections are injected as PE "filler" units inside the
    ACT(exp)-bound attention blocks, paced pro-rata per chunk
  - per-chunk software pipeline: scores for chunk kc+2 are emitted two
    deep so the PE always has independent work while exp(kc) runs
  - diagonal-chunk exps with off>=256 are split per head: skips the
    unused [512, 512+off) region and releases the scores PSUM sooner
  - engine budget: PE matmuls ~114us (the fp16 floor), ACT exp+copies
    ~88us, DVE ~54us, Pool ~32us (masks/memsets only)
fp8 DoubleRow was evaluated and rejected: e4m3 quantization of any
matmul site costs 2.9e-2..7e-2 relative error vs the 2e-2 gate.
"""

import numpy as np

import concourse.bass as bass
import concourse.mybir as mybir
from concourse.tile import TileContext
from concourse.vector_clock import ScopedClock
from concourse.bass_utils import run_bass_kernel_spmd

B, T, D = 2, 2048, 1024
H, DK = 16, 64
NCORES = 8
HPC = 4            # heads per core
QB = 512           # q block size
NQB = T // QB      # 4
NKC = T // 128     # 16 k-chunks
F16 = mybir.dt.float16
F32 = mybir.dt.float32
EXPF = mybir.ActivationFunctionType.Exp


class TC(TileContext):
    """This container's walrus only accepts one sync-wait per TPB_CTRL
    instruction; split the tile tail-drain waits into one nop each."""

    def _drain_and_barrier(self, tick_clock, wait_clock):
        carrier = self.nc.sync.nop(nofuse=True)
        wait_clock.add_sem_waits(
            carrier.ins, ScopedClock({None: tick_clock.global_clock})
        )
        si = carrier.ins.sync_info
        if si is not None and len(si.on_wait) > 1:
            waits = list(si.on_wait)
            carrier.ins.sync_info = mybir.SyncInfo(
                on_wait=[waits[0]], on_update=list(si.on_update)
            )
            for w in waits[1:]:
                nop = self.nc.sync.nop(nofuse=True)
                nop.ins.sync_info = mybir.SyncInfo(on_wait=[w], on_update=[])
        self.nc.sync.drain()
        self.nc.all_engine_barrier()
        assert self.sems is not None
        popped = self.nc._tile_sem_poison_stack.pop()
        assert popped is self._sem_poison
        self.nc.clear_and_free_semaphores(list(self.sems.allocated().values()))
        self.nc.all_engine_barrier()


def split_multi_waits(nc):
    """This walrus build accepts only one sync-wait per instruction; hoist
    extra waits onto single-wait NoOps inserted just before the instruction
    on the same engine."""
    for fn in nc.m.functions:
        for bb in fn.blocks:
            out = []
            for ins in bb.instructions:
                si = getattr(ins, "sync_info", None)
                is_isa = "ISA" in type(ins).__name__ or "PartitionBroadcast" in type(ins).__name__
                keep = 0 if is_isa else 1
                if si is not None and len(si.on_wait) > keep:
                    waits = list(si.on_wait)
                    keep_waits = waits[len(waits) - keep :] if keep else []
                    for i, w in enumerate(waits[: len(waits) - keep]):
                        out.append(
                            mybir.InstNoOp(
                                name=f"{ins.name}_w{i}",
                                engine=ins.engine,
                                sync_info=mybir.SyncInfo(on_wait=[w], on_update=[]),
                                bass_nofuse=True,
                            )
                        )
                    ins.sync_info = mybir.SyncInfo(
                        on_wait=keep_waits, on_update=list(si.on_update)
                    )
                out.append(ins)
            bb.instructions = out


def build_nc():
    nc = bass.Bass("TRN2", target_bir_lowering=False, debug=False)
    xT = nc.dram_tensor("xT", [D, T], F16, kind="ExternalInput")
    wqT = nc.dram_tensor("wqT", [128, 2048], F16, kind="ExternalInput")
    wkT = nc.dram_tensor("wkT", [128, 2048], F16, kind="ExternalInput")
    wvT = nc.dram_tensor("wvT", [128, 2048], F16, kind="ExternalInput")
    woT = nc.dram_tensor("woT", [128, 2048], F16, kind="ExternalInput")
    tmask = nc.dram_tensor("tmask", [128, 128], F16, kind="ExternalInput")
    sel65 = nc.dram_tensor("sel65", [65, 128], F16, kind="ExternalInput")
    yT = nc.dram_tensor("yT", [D, T], F16, kind="ExternalOutput")

    with TC(nc) as tc:
        with (
            tc.tile_pool(name="const", bufs=1) as cpool,
            tc.tile_pool(name="work", bufs=2) as wpool,
            tc.tile_pool(name="psA", bufs=2, space="PSUM") as psA,
            tc.tile_pool(name="psS", bufs=2, space="PSUM") as psS,
            tc.tile_pool(name="psV", bufs=1, space="PSUM") as psV,
        ):
            # ---- tiles ----
            xt = [cpool.tile([128, T], F16, tag=f"xt{kc}", name=f"xt{kc}") for kc in range(8)]
            wq = cpool.tile([128, 2048], F16, tag="wq", name="wq")
            wk = cpool.tile([128, 2048], F16, tag="wk", name="wk")
            wv = cpool.tile([128, 2048], F16, tag="wv", name="wv")
            wo = cpool.tile([128, 2048], F16, tag="wo", name="wo")
            maskt = cpool.tile([128, 128], F16, tag="mask")
            selt = cpool.tile([65, 128], F16, tag="sel")

            # warm the ACT exp table while DMAs run
            warm = cpool.tile([1, 8], F32, tag="warm")
            nc.vector.memset(warm[:], 0.0)
            nc.scalar.activation(warm[:], warm[:], EXPF, scale=1.0)

            # ---- input DMAs split across the two HWDGE queues (SP and
            # ACT): urgent first-projection inputs on SP, the rest on ACT ----
            def xf(kc):
                nc.sync.dma_start(
                    xt[kc][:, 0:QB], xT[128 * kc : 128 * (kc + 1), 0:QB]
                )

            nc.sync.dma_start(wq[:, 0:1024], wqT[:, 0:1024])
            xf(0)
            xf(1)
            nc.sync.dma_start(wq[:, 1024:2048], wqT[:, 1024:2048])
            xf(2)
            nc.sync.dma_start(wk[:], wkT[:, :])
            for kc in range(3, 8):
                xf(kc)
            nc.sync.dma_start(maskt[:], tmask[:, :])
            nc.sync.dma_start(selt[:], sel65[:, :])
            nc.sync.dma_start(wv[:], wvT[:, :])
            for kc in range(8):
                nc.sync.dma_start(
                    xt[kc][:, QB:T], xT[128 * kc : 128 * (kc + 1), QB:T]
                )
            nc.sync.dma_start(wo[:], woT[:, :])

            qt = [cpool.tile([128, T], F16, tag=f"qt{p}", name=f"qt{p}") for p in range(2)]
            kt = [cpool.tile([128, T], F16, tag=f"kt{p}", name=f"kt{p}") for p in range(2)]
            ao = [cpool.tile([128, T], F16, tag=f"ao{p}", name=f"ao{p}") for p in range(2)]
            vp = [
                [cpool.tile([128, 193], F16, tag=f"vp{tt}_{p}", name=f"vp{tt}_{p}") for p in range(2)]
                for tt in range(NKC)
            ]

            # ---- Q, K projections: out[p][:, jq] = W.T @ xT ----
            def qk_proj(p, wt, out_t, jq):
                ps = psA.tile([128, QB], F32, tag="psA", name=f"psqk{p}{jq}")
                for kc in range(8):
                    nc.tensor.matmul(
                        ps[:],
                        wt[:, 256 * kc + 128 * p : 256 * kc + 128 * (p + 1)],
                        xt[kc][:, QB * jq : QB * (jq + 1)],
                        start=(kc == 0),
                        stop=(kc == 7),
                    )
                nc.scalar.copy(out_t[p][:, QB * jq : QB * (jq + 1)], ps[:])

            def v_proj(tt):
                ps = psA.tile([128, QB], F32, tag="psA", name=f"psv{tt}")
                for kc in range(8):
                    nc.tensor.matmul(
                        ps[:, 0:256],
                        xt[kc][:, 128 * tt : 128 * (tt + 1)],
                        wv[:, 256 * kc : 256 * (kc + 1)],
                        start=(kc == 0),
                        stop=(kc == 7),
                    )
                for p in range(2):
                    v = vp[tt][p]
                    nc.gpsimd.memset(v[:, 64:66], 1.0)
                    nc.gpsimd.memset(v[:, 66:129], 0.0)
                    nc.vector.tensor_copy(v[:, 0:64], ps[:, 128 * p : 128 * p + 64])
                    nc.vector.tensor_copy(v[:, 129:193], ps[:, 128 * p + 64 : 128 * p + 128])

            # ---- output projection for one (q block, dim chunk) ----
            def out_proj_unit(j, et, tail=False):
                ps = psA.tile([128, QB], F32, tag="psA", name=f"pso{j}{et}")
                for p in range(2):
                    nc.tensor.matmul(
                        ps[:],
                        wo[:, 1024 * p + 128 * et : 1024 * p + 128 * (et + 1)],
                        ao[p][:, QB * j : QB * (j + 1)],
                        start=(p == 0),
                        stop=(p == 1),
                    )
                ysb = wpool.tile([128, QB], F16, tag="ysb", bufs=8, name="ysb")
                nc.vector.tensor_copy(ysb[:], ps[:])
                ring = nc.scalar if (tail and et % 2) else nc.sync
                ring.dma_start(
                    yT[128 * et : 128 * (et + 1), QB * j : QB * (j + 1)], ysb[:]
                )

            # reciprocal staging tiles: rows 1..63 must never hold NaN (the
            # sel matmul multiplies them by zero); zero them once up front
            rcbs = [
                cpool.tile([65, QB], F16, tag=f"rcb{i}", name=f"rcb{i}")
                for i in range(2)
            ]
            for t in rcbs:
                nc.gpsimd.memset(t[:], 0.0)
            rci = [0]

            # ---- attention over one q block; fillers are (weight, closure)
            # PE work consumed pro-rata across chunk steps; pre_units are
            # placed before the scores of a specific (p, kc) chunk ----
            def mk_scores(j):
                def scores(p, kc):
                    off = max(0, 128 * (kc - 4 * j))
                    ps = psS.tile([128, 1024], F32, tag="psS", name="psS")
                    for h in range(2):
                        nc.tensor.matmul(
                            ps[:, 512 * h + off : 512 * (h + 1)],
                            kt[p][64 * h : 64 * (h + 1), 128 * kc : 128 * (kc + 1)],
                            qt[p][64 * h : 64 * (h + 1), QB * j + off : QB * (j + 1)],
                            start=True,
                            stop=True,
                        )
                    return ps, off

                return scores

            def attention(j, fillers, pre_units, carry_in=None, next_first=None):
                scores = mk_scores(j)
                total_w = sum(w for w, _ in fillers) or 1
                done_w = 0
                fi = 0
                nch = 4 * j + 4
                steps = 2 * nch
                step = 0
                carry_out = [None]

                prefetched = [carry_in]
                for p in range(2):
                    P = wpool.tile([128, 1024 * NKC], F16, tag="P", bufs=3, name="P")
                    av0 = psV.tile([65, QB], F32, tag="av0", name="av0")
                    av1 = psV.tile([128, QB], F32, tag="av1", name="av1")
                    if prefetched[0] is not None:
                        queue = [prefetched[0]]
                        prefetched[0] = None
                    else:
                        for u in pre_units.pop((p, 0), ()):
                            u()
                        queue = [scores(p, 0)]
                    if nch > 1:
                        for u in pre_units.pop((p, 1), ()):
                            u()
                        queue.append(scores(p, 1))
                    for kc in range(nch):
                        ps, off = queue.pop(0)
                        if off >= 256:
                            # split per head: skips the unused [512, 512+off)
                            # region and frees the scores PSUM sooner
                            nc.scalar.activation(
                                P[:, 1024 * kc + off : 1024 * kc + 512],
                                ps[:, off:512],
                                EXPF,
                                scale=0.125,
                            )
                            nc.scalar.activation(
                                P[:, 1024 * kc + 512 + off : 1024 * (kc + 1)],
                                ps[:, 512 + off : 1024],
                                EXPF,
                                scale=0.125,
                            )
                        else:
                            nc.scalar.activation(
                                P[:, 1024 * kc + off : 1024 * (kc + 1)],
                                ps[:, off:1024],
                                EXPF,
                                scale=0.125,
                            )
                        if kc >= 4 * j:  # diagonal band: mask 128x128 blocks
                            for h in range(2):
                                sl = slice(
                                    1024 * kc + 512 * h + off,
                                    1024 * kc + 512 * h + off + 128,
                                )
                                nc.gpsimd.tensor_mul(P[:, sl], P[:, sl], maskt[:])
                        # next chunk's scores keep PE busy while ACT runs exp
                        if kc + 2 < nch:
                            for u in pre_units.pop((p, kc + 2), ()):
                                u()
                            queue.append(scores(p, kc + 2))
                        step += 1
                        if kc < nch - 1:
                            while fi < len(fillers) and done_w * steps < total_w * step:
                                done_w += fillers[fi][0]
                                fillers[fi][1]()
                                fi += 1
                        nc.tensor.matmul(
                            av0[:, off:QB],
                            vp[kc][p][:, 0:65],
                            P[:, 1024 * kc + off : 1024 * kc + 512],
                            start=(kc == 0),
                            stop=(kc == nch - 1),
                        )
                        nc.tensor.matmul(
                            av1[:, off:QB],
                            vp[kc][p][:, 65:193],
                            P[:, 1024 * kc + 512 + off : 1024 * (kc + 1)],
                            start=(kc == 0),
                            stop=(kc == nch - 1),
                        )
                    # softmax denominators: av0 row 64 = sum(P_h0),
                    # av1 row 0 = sum(P_h1); av1 values live on rows 64..127
                    rcb = rcbs[rci[0]]
                    rci[0] ^= 1
                    with nc.allow_low_precision("softmax denom recip in fp16"):
                        nc.vector.reciprocal(rcb[64:65, :], av0[64:65, :])
                        nc.vector.reciprocal(rcb[0:1, :], av1[0:1, :])
                    while fi < len(fillers) and done_w * steps < total_w * step:
                        done_w += fillers[fi][0]
                        fillers[fi][1]()
                        fi += 1
                    if p == 0:
                        # prefetch the next pair's first scores so PE has
                        # work while the reciprocal lands
                        for u in pre_units.pop((1, 0), ()):
                            u()
                        prefetched[0] = scores(1, 0)
                    if j == NQB - 1 and p == 1:
                        bc = psS.tile([128, QB], F32, tag="psS", name=f"bc{j}{p}")
                    else:
                        bc = psA.tile([128, QB], F32, tag="psA", name=f"bc{j}{p}")
                    nc.tensor.matmul(bc[:], selt[:], rcb[:], start=True, stop=True)
                    bcs = wpool.tile([128, QB], F32, tag="bcs", bufs=2, name="bcs")
                    nc.vector.tensor_copy(bcs[:], bc[:])
                    nc.vector.tensor_mul(
                        ao[p][0:64, QB * j : QB * (j + 1)], av0[0:64, :], bcs[0:64, :]
                    )
                    nc.vector.tensor_mul(
                        ao[p][64:128, QB * j : QB * (j + 1)],
                        av1[64:128, :],
                        bcs[64:128, :],
                    )
                for w, u in fillers[fi:]:
                    u()
                if next_first is not None:
                    carry_out[0] = next_first()
                return carry_out[0]

            # ---- main schedule: pair-0 projections first so attention(0)
            # can start after two projection calls ----
            for pp, wt, ot in ((0, wq, qt), (0, wk, kt), (1, wq, qt), (1, wk, kt)):
                qk_proj(pp, wt, ot, 0)
            carry = None
            first_scores = {jj: (lambda jj=jj: mk_scores(jj)(0, 0)) for jj in range(NQB)}
            for j in range(NQB):
                fillers = []
                if j + 1 < NQB:
                    jq = j + 1
                    for pp, wt, ot in ((0, wq, qt), (0, wk, kt), (1, wq, qt), (1, wk, kt)):
                        fillers.append(
                            (4096, lambda pp=pp, wt=wt, ot=ot, jq=jq: qk_proj(pp, wt, ot, jq))
                        )
                # out-proj of block 0 fills attention(1); blocks 1 and 2
                # both fill attention(3), which is the ACT-bound stretch
                op_blocks = {1: [0], 3: [1, 2]}.get(j, [])
                for jb in op_blocks:
                    for et in range(8):
                        fillers.append((1024, lambda jb=jb, et=et: out_proj_unit(jb, et)))
                pre_units = {}
                for tt in range(4 * j, 4 * j + 4):
                    pre_units.setdefault((0, max(0, tt - 1)), []).append(
                        lambda tt=tt: v_proj(tt)
                    )
                nf = (
                    (lambda jn=j + 1: first_scores[jn]())
                    if j + 1 < NQB
                    else None
                )
                carry = attention(j, fillers, pre_units, carry, nf)
            for et in range(8):
                out_proj_unit(NQB - 1, et, tail=True)
    split_multi_waits(nc)
    return nc


_NC = None


def _get_nc():
    global _NC
    if _NC is None:
        _NC = build_nc()
    return _NC


def kernel(x, W_q, W_k, W_v, W_o):
    x = np.asarray(x, dtype=np.float32)
    W_q = np.asarray(W_q, dtype=np.float32)
    W_k = np.asarray(W_k, dtype=np.float32)
    W_v = np.asarray(W_v, dtype=np.float32)
    W_o = np.asarray(W_o, dtype=np.float32)

    def pack8(a):  # [1024, 256] -> [128, 8*256], chunk kc at cols 256*kc
        return np.ascontiguousarray(
            a.reshape(8, 128, 256).transpose(1, 0, 2).reshape(128, 2048)
        ).astype(np.float16)

    def pack2(a):  # [256, 1024] -> [128, 2*1024], chunk p at cols 1024*p
        return np.ascontiguousarray(
            a.reshape(2, 128, 1024).transpose(1, 0, 2).reshape(128, 2048)
        ).astype(np.float16)

    tmask = np.triu(np.ones((128, 128), dtype=np.float16))
    sel65 = np.zeros((65, 128), dtype=np.float16)
    sel65[64, 0:64] = 1.0
    sel65[0, 64:128] = 1.0
    xTb = [np.ascontiguousarray(x[b].T).astype(np.float16) for b in range(B)]
    in_maps = []
    for c in range(NCORES):
        b, g = c // 4, c % 4
        hs = 256 * g
        in_maps.append(
            {
                "xT": xTb[b],
                "wqT": pack8(W_q[hs : hs + 256, :].T),
                "wkT": pack8(W_k[hs : hs + 256, :].T),
                "wvT": pack8(W_v[hs : hs + 256, :].T),
                "woT": pack2(W_o[:, hs : hs + 256].T),
                "tmask": tmask,
                "sel65": sel65,
            }
        )
    res = run_bass_kernel_spmd(_get_nc(), in_maps, core_ids=list(range(NCORES)))
    out = np.empty((B, T, D), dtype=np.float32)
    for b in range(B):
        acc = res.results[4 * b]["yT"].astype(np.float32)
        for g in range(1, 4):
            acc = acc + res.results[4 * b + g]["yT"]
        out[b] = acc.T
    return out


# revision 80
# speedup vs baseline: 1.0722x; 1.0030x over previous
"""Causal self-attention Trainium2 kernel (B=2, T=2048, D=1024, H=16).

Sharding: 8 cores = 2 batch groups x 4 head groups; each core computes
batch b = c//4, heads 4*(c%4)..4*(c%4)+3 (256 QKV dims), and a partial
output projection y_cT = W_o[:, slice] @ attnout (contribution summed on
host across the 4 cores of each batch group).

All on-device compute in fp16 operands with fp32 PSUM accumulation.
Everything is kept "transposed" ([dim, seq]) so no on-device transposes
are needed:
  QT/KT = W @ xT               [256, 2048]
  V     = x @ WvT              [2048, 256]   (seq on partitions)
  ST[k,q] = sum_d K[k,d]Q[q,d] (k on partitions, q streaming)
  P = exp(ST/8); causal mask applied multiplicatively on the diagonal
  128x128 blocks (Pool engine, SBUF only - Pool cannot touch PSUM)
  avT[d,q] = sum_k [V|1][k,d] P[k,q]  -> row of ones gives softmax denom
  attnout[d,q] = avT * (1/denom); the reciprocal is broadcast across
  partitions with a small selector matmul (PE)
  yT[e,q] = WoT.T @ attnout  (partial over this core's 256 dims)

Schedule highlights (all found against the TimelineSim cost model):
  - input DMAs are split and priority-ordered (weights host-packed to
    [128, 2048] so each loads in one descriptor); the first projection
    starts ~4us in
  - V projections, the next block's Q/K projections and earlier blocks'
    output projections are injected as PE "filler" units inside the
    ACT(exp)-bound attention blocks, paced pro-rata per chunk
  - per-chunk software pipeline: scores for chunk kc+2 are emitted two
    deep so the PE always has independent work while exp(kc) runs
  - diagonal-chunk exps with off>=256 are split per head: skips the
    unused [512, 512+off) region and releases the scores PSUM sooner
  - engine budget: PE matmuls ~114us (the fp16 floor), ACT exp+copies
    ~88us, DVE ~54us, Pool ~32us (masks/memsets only)
fp8 DoubleRow was evaluated and rejected: e4m3 quantization of any
matmul site costs 2.9e-2..7e-2 relative error vs the 2e-2 gate.
"""

import numpy as np

import concourse.bass as bass
import concourse.mybir as mybir
from concourse.tile import TileContext
from concourse.vector_clock import ScopedClock
from concourse.bass_utils import run_bass_kernel_spmd

B, T, D = 2, 2048, 1024
H, DK = 16, 64
NCORES = 8
HPC = 4            # heads per core
QB = 512           # q block size
NQB = T // QB      # 4
NKC = T // 128     # 16 k-chunks
F16 = mybir.dt.float16
F32 = mybir.dt.float32
EXPF = mybir.ActivationFunctionType.Exp


class TC(TileContext):
    """This container's walrus only accepts one sync-wait per TPB_CTRL
    instruction; split the tile tail-drain waits into one nop each."""

    def _drain_and_barrier(self, tick_clock, wait_clock):
        carrier = self.nc.sync.nop(nofuse=True)
        wait_clock.add_sem_waits(
            carrier.ins, ScopedClock({None: tick_clock.global_clock})
        )
        si = carrier.ins.sync_info
        if si is not None and len(si.on_wait) > 1:
            waits = list(si.on_wait)
            carrier.ins.sync_info = mybir.SyncInfo(
                on_wait=[waits[0]], on_update=list(si.on_update)
            )
            for w in waits[1:]:
                nop = self.nc.sync.nop(nofuse=True)
                nop.ins.sync_info = mybir.SyncInfo(on_wait=[w], on_update=[])
        self.nc.sync.drain()
        self.nc.all_engine_barrier()
        assert self.sems is not None
        popped = self.nc._tile_sem_poison_stack.pop()
        assert popped is self._sem_poison
        self.nc.clear_and_free_semaphores(list(self.sems.allocated().values()))
        self.nc.all_engine_barrier()


def split_multi_waits(nc):
    """This walrus build accepts only one sync-wait per instruction; hoist
    extra waits onto single-wait NoOps inserted just before the instruction
    on the same engine."""
    for fn in nc.m.functions:
        for bb in fn.blocks:
            out = []
            for ins in bb.instructions:
                si = getattr(ins, "sync_info", None)
                is_isa = "ISA" in type(ins).__name__ or "PartitionBroadcast" in type(ins).__name__
                keep = 0 if is_isa else 1
                if si is not None and len(si.on_wait) > keep:
                    waits = list(si.on_wait)
                    keep_waits = waits[len(waits) - keep :] if keep else []
                    for i, w in enumerate(waits[: len(waits) - keep]):
                        out.append(
                            mybir.InstNoOp(
                                name=f"{ins.name}_w{i}",
                                engine=ins.engine,
                                sync_info=mybir.SyncInfo(on_wait=[w], on_update=[]),
                                bass_nofuse=True,
                            )
                        )
                    ins.sync_info = mybir.SyncInfo(
                        on_wait=keep_waits, on_update=list(si.on_update)
                    )
                out.append(ins)
            bb.instructions = out


def build_nc():
    nc = bass.Bass("TRN2", target_bir_lowering=False, debug=False)
    xT = nc.dram_tensor("xT", [D, T], F16, kind="ExternalInput")
    wqT = nc.dram_tensor("wqT", [128, 2048], F16, kind="ExternalInput")
    wkT = nc.dram_tensor("wkT", [128, 2048], F16, kind="ExternalInput")
    wvT = nc.dram_tensor("wvT", [128, 2048], F16, kind="ExternalInput")
    woT = nc.dram_tensor("woT", [128, 2048], F16, kind="ExternalInput")
    tmask = nc.dram_tensor("tmask", [128, 128], F16, kind="ExternalInput")
    sel65 = nc.dram_tensor("sel65", [65, 128], F16, kind="ExternalInput")
    yT = nc.dram_tensor("yT", [D, T], F16, kind="ExternalOutput")

    with TC(nc) as tc:
        with (
            tc.tile_pool(name="const", bufs=1) as cpool,
            tc.tile_pool(name="work", bufs=2) as wpool,
            tc.tile_pool(name="psA", bufs=2, space="PSUM") as psA,
            tc.tile_pool(name="psS", bufs=2, space="PSUM") as psS,
            tc.tile_pool(name="psV", bufs=1, space="PSUM") as psV,
        ):
            # ---- tiles ----
            xt = [cpool.tile([128, T], F16, tag=f"xt{kc}", name=f"xt{kc}") for kc in range(8)]
            wq = cpool.tile([128, 2048], F16, tag="wq", name="wq")
            wk = cpool.tile([128, 2048], F16, tag="wk", name="wk")
            wv = cpool.tile([128, 2048], F16, tag="wv", name="wv")
            wo = cpool.tile([128, 2048], F16, tag="wo", name="wo")
            maskt = cpool.tile([128, 128], F16, tag="mask")
            selt = cpool.tile([65, 128], F16, tag="sel")

            # warm the ACT exp table while DMAs run
            warm = cpool.tile([1, 8], F32, tag="warm")
            nc.vector.memset(warm[:], 0.0)
            nc.scalar.activation(warm[:], warm[:], EXPF, scale=1.0)

            # ramp the PE p-state before the first real matmul arrives
            # (~4us in, behind the priority DMAs): a chain of matmuls on
            # zeroed SBUF keeps the clock ramping from t~0.4us so the
            # projection pre-loop runs at full speed
            zwarm = cpool.tile([128, 256], F16, tag="zwarm", name="zwarm")
            nc.gpsimd.memset(zwarm[:], 0.0)
            pswarm = psA.tile([128, 256], F32, tag="psA", name="pswarm")
            for wi in range(16):
                nc.tensor.matmul(
                    pswarm[:],
                    zwarm[:, 0:128],
                    zwarm[:],
                    start=(wi == 0),
                    stop=(wi == 15),
                )

            # ---- input DMAs split across the two HWDGE queues (SP and
            # ACT): urgent first-projection inputs on SP, the rest on ACT ----
            def xf(kc):
                nc.sync.dma_start(
                    xt[kc][:, 0:QB], xT[128 * kc : 128 * (kc + 1), 0:QB]
                )

            nc.sync.dma_start(wq[:, 0:1024], wqT[:, 0:1024])
            xf(0)
            xf(1)
            nc.sync.dma_start(wq[:, 1024:2048], wqT[:, 1024:2048])
            xf(2)
            nc.sync.dma_start(wk[:], wkT[:, :])
            for kc in range(3, 8):
                xf(kc)
            nc.sync.dma_start(maskt[:], tmask[:, :])
            nc.sync.dma_start(selt[:], sel65[:, :])
            nc.sync.dma_start(wv[:], wvT[:, :])
            for kc in range(8):
                nc.sync.dma_start(
                    xt[kc][:, QB:T], xT[128 * kc : 128 * (kc + 1), QB:T]
                )
            nc.sync.dma_start(wo[:], woT[:, :])

            qt = [cpool.tile([128, T], F16, tag=f"qt{p}", name=f"qt{p}") for p in range(2)]
            kt = [cpool.tile([128, T], F16, tag=f"kt{p}", name=f"kt{p}") for p in range(2)]
            ao = [cpool.tile([128, T], F16, tag=f"ao{p}", name=f"ao{p}") for p in range(2)]
            vp = [
                [cpool.tile([128, 193], F16, tag=f"vp{tt}_{p}", name=f"vp{tt}_{p}") for p in range(2)]
                for tt in range(NKC)
            ]

            # ---- Q, K projections: out[p][:, jq] = W.T @ xT ----
            def qk_proj(p, wt, out_t, jq):
                ps = psA.tile([128, QB], F32, tag="psA", name=f"psqk{p}{jq}")
                for kc in range(8):
                    nc.tensor.matmul(
                        ps[:],
                        wt[:, 256 * kc + 128 * p : 256 * kc + 128 * (p + 1)],
                        xt[kc][:, QB * jq : QB * (jq + 1)],
                        start=(kc == 0),
                        stop=(kc == 7),
                    )
                nc.scalar.copy(out_t[p][:, QB * jq : QB * (jq + 1)], ps[:])

            def v_proj(tt):
                ps = psA.tile([128, QB], F32, tag="psA", name=f"psv{tt}")
                for kc in range(8):
                    nc.tensor.matmul(
                        ps[:, 0:256],
                        xt[kc][:, 128 * tt : 128 * (tt + 1)],
                        wv[:, 256 * kc : 256 * (kc + 1)],
                        start=(kc == 0),
                        stop=(kc == 7),
                    )
                for p in range(2):
                    v = vp[tt][p]
                    nc.gpsimd.memset(v[:, 64:66], 1.0)
                    nc.gpsimd.memset(v[:, 66:129], 0.0)
                    nc.vector.tensor_copy(v[:, 0:64], ps[:, 128 * p : 128 * p + 64])
                    nc.vector.tensor_copy(v[:, 129:193], ps[:, 128 * p + 64 : 128 * p + 128])

            # ---- output projection for one (q block, dim chunk) ----
            def out_proj_unit(j, et, tail=False):
                ps = psA.tile([128, QB], F32, tag="psA", name=f"pso{j}{et}")
                for p in range(2):
                    nc.tensor.matmul(
                        ps[:],
                        wo[:, 1024 * p + 128 * et : 1024 * p + 128 * (et + 1)],
                        ao[p][:, QB * j : QB * (j + 1)],
                        start=(p == 0),
                        stop=(p == 1),
                    )
                ysb = wpool.tile([128, QB], F16, tag="ysb", bufs=8, name="ysb")
                nc.vector.tensor_copy(ysb[:], ps[:])
                ring = nc.scalar if (tail and et % 2) else nc.sync
                ring.dma_start(
                    yT[128 * et : 128 * (et + 1), QB * j : QB * (j + 1)], ysb[:]
                )

            # reciprocal staging tiles: rows 1..63 must never hold NaN (the
            # sel matmul multiplies them by zero); zero them once up front
            rcbs = [
                cpool.tile([65, QB], F16, tag=f"rcb{i}", name=f"rcb{i}")
                for i in range(2)
            ]
            for t in rcbs:
                nc.gpsimd.memset(t[:], 0.0)
            rci = [0]

            # ---- attention over one q block; fillers are (weight, closure)
            # PE work consumed pro-rata across chunk steps; pre_units are
            # placed before the scores of a specific (p, kc) chunk ----
            def mk_scores(j):
                def scores(p, kc):
                    off = max(0, 128 * (kc - 4 * j))
                    ps = psS.tile([128, 1024], F32, tag="psS", name="psS")
                    for h in range(2):
                        nc.tensor.matmul(
                            ps[:, 512 * h + off : 512 * (h + 1)],
                            kt[p][64 * h : 64 * (h + 1), 128 * kc : 128 * (kc + 1)],
                            qt[p][64 * h : 64 * (h + 1), QB * j + off : QB * (j + 1)],
                            start=True,
                            stop=True,
                        )
                    return ps, off

                return scores

            def attention(j, fillers, pre_units, carry_in=None, next_first=None):
                scores = mk_scores(j)
                total_w = sum(w for w, _ in fillers) or 1
                done_w = 0
                fi = 0
                nch = 4 * j + 4
                steps = 2 * nch
                step = 0
                carry_out = [None]

                prefetched = [carry_in]
                for p in range(2):
                    P = wpool.tile([128, 1024 * NKC], F16, tag="P", bufs=3, name="P")
                    av0 = psV.tile([65, QB], F32, tag="av0", name="av0")
                    av1 = psV.tile([128, QB], F32, tag="av1", name="av1")
                    if prefetched[0] is not None:
                        queue = [prefetched[0]]
                        prefetched[0] = None
                    else:
                        for u in pre_units.pop((p, 0), ()):
                            u()
                        queue = [scores(p, 0)]
                    if nch > 1:
                        for u in pre_units.pop((p, 1), ()):
                            u()
                        queue.append(scores(p, 1))
                    for kc in range(nch):
                        ps, off = queue.pop(0)
                        if off >= 256:
                            # split per head: skips the unused [512, 512+off)
                            # region and frees the scores PSUM sooner
                            nc.scalar.activation(
                                P[:, 1024 * kc + off : 1024 * kc + 512],
                                ps[:, off:512],
                                EXPF,
                                scale=0.125,
                            )
                            nc.scalar.activation(
                                P[:, 1024 * kc + 512 + off : 1024 * (kc + 1)],
                                ps[:, 512 + off : 1024],
                                EXPF,
                                scale=0.125,
                            )
                        else:
                            nc.scalar.activation(
                                P[:, 1024 * kc + off : 1024 * (kc + 1)],
                                ps[:, off:1024],
                                EXPF,
                                scale=0.125,
                            )
                        if kc >= 4 * j:  # diagonal band: mask 128x128 blocks
                            for h in range(2):
                                sl = slice(
                                    1024 * kc + 512 * h + off,
                                    1024 * kc + 512 * h + off + 128,
                                )
                                nc.gpsimd.tensor_mul(P[:, sl], P[:, sl], maskt[:])
                        # next chunk's scores keep PE busy while ACT runs exp
                        if kc + 2 < nch:
                            for u in pre_units.pop((p, kc + 2), ()):
                                u()
                            queue.append(scores(p, kc + 2))
                        step += 1
                        if kc < nch - 1:
                            while fi < len(fillers) and done_w * steps < total_w * step:
                                done_w += fillers[fi][0]
                                fillers[fi][1]()
                                fi += 1
                        nc.tensor.matmul(
                            av0[:, off:QB],
                            vp[kc][p][:, 0:65],
                            P[:, 1024 * kc + off : 1024 * kc + 512],
                            start=(kc == 0),
                            stop=(kc == nch - 1),
                        )
                        nc.tensor.matmul(
                            av1[:, off:QB],
                            vp[kc][p][:, 65:193],
                            P[:, 1024 * kc + 512 + off : 1024 * (kc + 1)],
                            start=(kc == 0),
                            stop=(kc == nch - 1),
                        )
                    # softmax denominators: av0 row 64 = sum(P_h0),
                    # av1 row 0 = sum(P_h1); av1 values live on rows 64..127
                    rcb = rcbs[rci[0]]
                    rci[0] ^= 1
                    with nc.allow_low_precision("softmax denom recip in fp16"):
                        nc.vector.reciprocal(rcb[64:65, :], av0[64:65, :])
                        nc.vector.reciprocal(rcb[0:1, :], av1[0:1, :])
                    while fi < len(fillers) and done_w * steps < total_w * step:
                        done_w += fillers[fi][0]
                        fillers[fi][1]()
                        fi += 1
                    if p == 0:
                        # prefetch the next pair's first scores so PE has
                        # work while the reciprocal lands
                        for u in pre_units.pop((1, 0), ()):
                            u()
                        prefetched[0] = scores(1, 0)
                    if j == NQB - 1 and p == 1:
                        bc = psS.tile([128, QB], F32, tag="psS", name=f"bc{j}{p}")
                    else:
                        bc = psA.tile([128, QB], F32, tag="psA", name=f"bc{j}{p}")
                    nc.tensor.matmul(bc[:], selt[:], rcb[:], start=True, stop=True)
                    bcs = wpool.tile([128, QB], F32, tag="bcs", bufs=2, name="bcs")
                    nc.vector.tensor_copy(bcs[:], bc[:])
                    nc.vector.tensor_mul(
                        ao[p][0:64, QB * j : QB * (j + 1)], av0[0:64, :], bcs[0:64, :]
                    )
                    nc.vector.tensor_mul(
                        ao[p][64:128, QB * j : QB * (j + 1)],
                        av1[64:128, :],
                        bcs[64:128, :],
                    )
                for w, u in fillers[fi:]:
                    u()
                if next_first is not None:
                    carry_out[0] = next_first()
                return carry_out[0]

            # ---- main schedule: pair-0 projections first so attention(0)
            # can start after two projection calls ----
            for pp, wt, ot in ((0, wq, qt), (0, wk, kt), (1, wq, qt), (1, wk, kt)):
                qk_proj(pp, wt, ot, 0)
            carry = None
            first_scores = {jj: (lambda jj=jj: mk_scores(jj)(0, 0)) for jj in range(NQB)}
            for j in range(NQB):
                fillers = []
                if j + 1 < NQB:
                    jq = j + 1
                    for pp, wt, ot in ((0, wq, qt), (0, wk, kt), (1, wq, qt), (1, wk, kt)):
                        fillers.append(
                            (4096, lambda pp=pp, wt=wt, ot=ot, jq=jq: qk_proj(pp, wt, ot, jq))
                        )
                # out-proj of block 0 fills attention(1); blocks 1 and 2
                # both fill attention(3), which is the ACT-bound stretch
                op_blocks = {1: [0], 3: [1, 2]}.get(j, [])
                for jb in op_blocks:
                    for et in range(8):
                        fillers.append((1024, lambda jb=jb, et=et: out_proj_unit(jb, et)))
                pre_units = {}
                for tt in range(4 * j, 4 * j + 4):
                    pre_units.setdefault((0, max(0, tt - 1)), []).append(
                        lambda tt=tt: v_proj(tt)
                    )
                nf = (
                    (lambda jn=j + 1: first_scores[jn]())
                    if j + 1 < NQB
                    else None
                )
                carry = attention(j, fillers, pre_units, carry, nf)
            for et in range(8):
                out_proj_unit(NQB - 1, et, tail=True)
    split_multi_waits(nc)
    return nc


_NC = None


def _get_nc():
    global _NC
    if _NC is None:
        _NC = build_nc()
    return _NC


def kernel(x, W_q, W_k, W_v, W_o):
    x = np.asarray(x, dtype=np.float32)
    W_q = np.asarray(W_q, dtype=np.float32)
    W_k = np.asarray(W_k, dtype=np.float32)
    W_v = np.asarray(W_v, dtype=np.float32)
    W_o = np.asarray(W_o, dtype=np.float32)

    def pack8(a):  # [1024, 256] -> [128, 8*256], chunk kc at cols 256*kc
        return np.ascontiguousarray(
            a.reshape(8, 128, 256).transpose(1, 0, 2).reshape(128, 2048)
        ).astype(np.float16)

    def pack2(a):  # [256, 1024] -> [128, 2*1024], chunk p at cols 1024*p
        return np.ascontiguousarray(
            a.reshape(2, 128, 1024).transpose(1, 0, 2).reshape(128, 2048)
        ).astype(np.float16)

    tmask = np.triu(np.ones((128, 128), dtype=np.float16))
    sel65 = np.zeros((65, 128), dtype=np.float16)
    sel65[64, 0:64] = 1.0
    sel65[0, 64:128] = 1.0
    xTb = [np.ascontiguousarray(x[b].T).astype(np.float16) for b in range(B)]
    in_maps = []
    for c in range(NCORES):
        b, g = c // 4, c % 4
        hs = 256 * g
        in_maps.append(
            {
                "xT": xTb[b],
                "wqT": pack8(W_q[hs : hs + 256, :].T),
                "wkT": pack8(W_k[hs : hs + 256, :].T),
                "wvT": pack8(W_v[hs : hs + 256, :].T),
                "woT": pack2(W_o[:, hs : hs + 256].T),
                "tmask": tmask,
                "sel65": sel65,
            }
        )
    res = run_bass_kernel_spmd(_get_nc(), in_maps, core_ids=list(range(NCORES)))
    out = np.empty((B, T, D), dtype=np.float32)
    for b in range(B):
        acc = res.results[4 * b]["yT"].astype(np.float32)
        for g in range(1, 4):
            acc = acc + res.results[4 * b + g]["yT"]
        out[b] = acc.T
    return out
